# revision 46
# baseline (speedup 1.0000x reference)
"""TRN2 Bass kernel for nn_BlockMoVaE (attention + MoE/VE routing block).

Self-contained: accepts FULL inputs, shards across 8 NeuronCores, returns
FULL output.

Three SPMD launches with host re-layout between them (host does only data
movement / routing; all FLOPs stay on device).  The router top-2 decision
is discrete, so everything feeding the logits (all of phase 1) runs at
f32r precision; only the post-routing expert MLP uses bf16/fp8.

  L1a  (token-parallel, 512 tokens/core = 2 causally-balanced 256-strips):
       x-rms stats, raw-x f32r QKV projections, token-major rope +
       per-head rmsnorm (the per-token x-norm scalar cancels inside the
       head rmsnorm, so Q/K project raw x; V is scaled by r explicitly).
       Exports token-major q/k/v (f32).
  L1b  (token-parallel attention): host re-lays q/k/v into score-friendly
       f32r layouts.  Causal work is balanced by giving core ci the
       256-token strips {ci, 7-ci} of its batch; the static program
       computes (8, 16) key-128-slots for the two strips; fully-dead
       slots are killed by a rank-1 bias row (-30000) folded into the
       score matmul contraction, diagonal tiles by static affine_selects.
       Softmax denominators ride along as a ones-column of V.  Then
       wo + residual + xf rmsnorm + router logits.
  L2   (expert-parallel MoE): host routes top-2 and gathers tokens per
       expert with sqrt(gate) pre-scaling (relu^2 is 2-homogeneous so the
       gate factors exactly); bf16 up / fp8-DoubleRow down projections;
       VE rows host-gathered, gate-pre-scaled, summed on device.
"""
import numpy as np
import ml_dtypes

import concourse.bass as bass
import concourse.bacc as bacc
import concourse.mybir as mybir
import concourse.tile as tile
from concourse.bass_utils import run_bass_kernel_spmd
from concourse.alu_op_type import AluOpType
from contextlib import ExitStack
from collections import deque

# ---- problem constants (hardcoded per contest rules) ----
B, T, C = 2, 2048, 1024
NH, NKV, HD = 16, 8, 64
E_MLP, E_VE, TOPK = 8, 2, 2
HID = 2048
VOCAB = 50257
EPS = 1e-6
NCORES = 8
S = 512              # tokens per core
QT = 4               # 128-token tiles per core (2 strips of 256)
NSTRIP = 2
SW = 256             # strip width (queries)
NPOS = (8, 16)       # static key-slot count per strip
POS_BASE = (0, 8)    # slot base in kT layout (total 24)
NSLOT_TOT = 24
DEAD_BIAS = -30000.0
NCAP0 = 1024

f32 = mybir.dt.float32
f32r = mybir.dt.float32r
bf16 = mybir.dt.bfloat16
fp8e4 = mybir.dt.float8e4
AF = mybir.ActivationFunctionType
DR = mybir.MatmulPerfMode.DoubleRow
E4 = ml_dtypes.float8_e4m3
BF = ml_dtypes.bfloat16

_prog_cache = {}


def _register_consts(nc, values):
    for value in values:
        key = (f32, float(value))
        if key not in nc.const_aps.aps:
            t = nc.alloc_sbuf_tensor(f"constap-{value}", [128, 1], f32)
            nc.gpsimd.memset(t.ap(), float(value))
            nc.const_aps.aps[key] = t.ap()
    nc.all_engine_barrier()


# --------------------------------------------------------------------------
# L1a: x stats + QKV projection + rope + head-rms (token-major epilogues)
# --------------------------------------------------------------------------
def build_1a():
    nc = bacc.Bacc("TRN2", target_bir_lowering=False, debug=False,
                   num_devices=NCORES)

    x_fm = nc.dram_tensor("x_fm", [128, 8, S], f32r, kind="ExternalInput").ap()
    x_tm = nc.dram_tensor("x_tm", [128, QT, C], f32, kind="ExternalInput").ap()
    wq_t = nc.dram_tensor("wq_t", [128, 8, NH * HD], f32r,
                          kind="ExternalInput").ap()
    wk_t = nc.dram_tensor("wk_t", [128, 8, NKV * HD], f32r,
                          kind="ExternalInput").ap()
    wv_t = nc.dram_tensor("wv_t", [128, 8, NKV * HD], f32r,
                          kind="ExternalInput").ap()
    cos_tm = nc.dram_tensor("cos_tm", [128, QT, 32], f32,
                            kind="ExternalInput").ap()
    sin_tm = nc.dram_tensor("sin_tm", [128, QT, 32], f32,
                            kind="ExternalInput").ap()

    q_out = nc.dram_tensor("q_out", [128, QT, NH * HD], f32,
                           kind="ExternalOutput").ap()
    k_out = nc.dram_tensor("k_out", [128, QT, NKV * HD], f32,
                           kind="ExternalOutput").ap()
    v_out = nc.dram_tensor("v_out", [128, QT, NKV * HD], f32,
                           kind="ExternalOutput").ap()

    _register_consts(nc, [EPS])
    with tile.TileContext(nc) as tc, ExitStack() as est:
        wp = est.enter_context(tc.tile_pool(name="wp", bufs=1))
        work = est.enter_context(tc.tile_pool(name="work", bufs=2))
        outp = est.enter_context(tc.tile_pool(name="outp", bufs=1))
        ps_q = est.enter_context(tc.tile_pool(name="ps_q", bufs=2, space="PSUM"))
        ps_k = est.enter_context(tc.tile_pool(name="ps_k", bufs=2, space="PSUM"))
        ps_v = est.enter_context(tc.tile_pool(name="ps_v", bufs=2, space="PSUM"))

        xf = wp.tile([128, 8, S], f32r, name="xf")
        xt = wp.tile([128, QT, C], f32, name="xt")
        wq = wp.tile([128, 8, NH * HD], f32r, name="wq")
        wk = wp.tile([128, 8, NKV * HD], f32r, name="wk")
        wv = wp.tile([128, 8, NKV * HD], f32r, name="wv")
        cs = wp.tile([128, QT, 32], f32, name="cs")
        sn = wp.tile([128, QT, 32], f32, name="sn")
        nc.sync.dma_start(xf[:], x_fm[:])
        for cc in range(8):
            nc.sync.dma_start(wq[:, cc, :], wq_t[:, cc, :])
            nc.sync.dma_start(wk[:, cc, :], wk_t[:, cc, :])
            nc.sync.dma_start(wv[:, cc, :], wv_t[:, cc, :])
        nc.sync.dma_start(cs[:], cos_tm[:])
        nc.sync.dma_start(sn[:], sin_tm[:])
        nc.sync.dma_start(xt[:], x_tm[:])

        qe = outp.tile([128, QT, NH * HD], f32, name="qe")
        ke = outp.tile([128, QT, NKV * HD], f32, name="ke")
        ve = outp.tile([128, QT, NKV * HD], f32, name="ve")

        def rope_norm(ps, nh, t, out_tile):
            """Token-major rope + per-head rmsnorm from psum [128, nh*64]."""
            qs = work.tile([128, nh, HD], f32, tag=f"qs{nh}", name="qs")
            nc.scalar.copy(qs[:], ps[:].rearrange("p (h d) -> p h d", d=HD))
            cosb = cs[:, t:t + 1, :].broadcast_to([128, nh, 32])
            sinb = sn[:, t:t + 1, :].broadcast_to([128, nh, 32])
            rp = work.tile([128, nh, HD], f32, tag=f"rp{nh}", name="rp")
            a = work.tile([128, nh, 32], f32, tag=f"ra{nh}", name="ra")
            b = work.tile([128, nh, 32], f32, tag=f"rb{nh}", name="rb")
            c2 = work.tile([128, nh, 32], f32, tag=f"rc{nh}", name="rc")
            d2 = work.tile([128, nh, 32], f32, tag=f"rd{nh}", name="rd")
            nc.vector.tensor_mul(a[:], qs[:, :, 0:32], cosb)
            nc.vector.tensor_mul(b[:], qs[:, :, 32:64], sinb)
            nc.gpsimd.tensor_mul(c2[:], qs[:, :, 32:64], cosb)
            nc.gpsimd.tensor_mul(d2[:], qs[:, :, 0:32], sinb)
            nc.gpsimd.tensor_add(rp[:, :, 0:32], a[:], b[:])
            nc.vector.tensor_sub(rp[:, :, 32:64], c2[:], d2[:])
            sq = work.tile([128, nh, HD], f32, tag=f"sq{nh}", name="sq")
            nc.scalar.activation(sq[:], rp[:], AF.Square)
            hs = work.tile([128, nh, 1], f32, tag=f"hs{nh}", name="hs")
            nc.vector.tensor_reduce(out=hs[:], in_=sq[:], op=AluOpType.add,
                                    axis=mybir.AxisListType.X)
            sh = work.tile([128, nh, 1], f32, tag=f"sh{nh}", name="sh")
            nc.scalar.activation(sh[:], hs[:], AF.Sqrt, bias=EPS,
                                 scale=1.0 / HD)
            rh = work.tile([128, nh, 1], f32, tag=f"rh{nh}", name="rh")
            with nc.allow_low_precision(reason="head rms recip"):
                nc.vector.reciprocal(rh[:], sh[:])
            nc.vector.tensor_mul(
                out_tile[:].rearrange("p (h d) -> p h d", d=HD),
                rp[:], rh[:].broadcast_to([128, nh, HD]))

        for t in range(QT):
            # per-token inv-rms of x (V only; cancels inside Q/K head-rms)
            xsq = work.tile([128, C], f32, tag="xsq", name="xsq")
            nc.scalar.activation(xsq[:], xt[:, t, :], AF.Square)
            ssq = work.tile([128, 1], f32, tag="ssq", name="ssq")
            nc.vector.tensor_reduce(out=ssq[:], in_=xsq[:], op=AluOpType.add,
                                    axis=mybir.AxisListType.XYZW)
            sx = work.tile([128, 1], f32, tag="sx", name="sx")
            nc.scalar.activation(sx[:], ssq[:], AF.Sqrt, bias=EPS,
                                 scale=1.0 / C)
            r = work.tile([128, 1], f32, tag="r", name="r")
            with nc.allow_low_precision(reason="x rms recip"):
                nc.vector.reciprocal(r[:], sx[:])

            q_ps = ps_q.tile([128, NH * HD], f32, name="q_ps")
            k_ps = ps_k.tile([128, NKV * HD], f32, name="k_ps")
            v_ps = ps_v.tile([128, NKV * HD], f32, name="v_ps")
            for half in range(2):
                hsl = bass.ts(half, NH * HD // 2)
                for cc in range(8):
                    nc.tensor.matmul(q_ps[:, hsl],
                                     xf[:, cc, bass.ts(t, 128)],
                                     wq[:, cc, hsl],
                                     start=(cc == 0), stop=(cc == 7))
            for cc in range(8):
                nc.tensor.matmul(k_ps[:], xf[:, cc, bass.ts(t, 128)],
                                 wk[:, cc, :], start=(cc == 0), stop=(cc == 7))
            for cc in range(8):
                nc.tensor.matmul(v_ps[:], xf[:, cc, bass.ts(t, 128)],
                                 wv[:, cc, :], start=(cc == 0), stop=(cc == 7))

            rope_norm(q_ps, NH, t, qe[:, t, :])
            nc.sync.dma_start(q_out[:, t, :], qe[:, t, :])
            rope_norm(k_ps, NKV, t, ke[:, t, :])
            nc.sync.dma_start(k_out[:, t, :], ke[:, t, :])
            nc.vector.tensor_scalar_mul(ve[:, t, :], v_ps[:], r[:])
            nc.sync.dma_start(v_out[:, t, :], ve[:, t, :])

    nc.compile()
    return nc


# --------------------------------------------------------------------------
# L1b: attention + wo + residual + xf rmsnorm + router logits
# --------------------------------------------------------------------------
def build_1b(masked: bool):
    nc = bacc.Bacc("TRN2", target_bir_lowering=False, debug=False,
                   num_devices=NCORES)

    # q rows 0..63 = head dims, row 64 = 1.0 (rank-1 bias carrier)
    q_sc = nc.dram_tensor("q_sc", [65, NH, NSTRIP, SW], f32r,
                          kind="ExternalInput").ap()
    kt_sc = nc.dram_tensor("kt_sc", [65, NKV, NSLOT_TOT, 128], f32r,
                           kind="ExternalInput").ap()
    # v columns 0..63 = v dims, col 64 = 1.0 (softmax denominator)
    v_sc = nc.dram_tensor("v_sc", [128, NKV, NSLOT_TOT, 65], f32r,
                          kind="ExternalInput").ap()
    x_fm32 = nc.dram_tensor("x_fm32", [128, 8, S], f32,
                            kind="ExternalInput").ap()
    wo_sc = nc.dram_tensor("wo_sc", [128, 8, C], f32r,
                           kind="ExternalInput").ap()
    rw_sb = nc.dram_tensor("rw_sb", [128, 8, E_MLP + E_VE], f32,
                           kind="ExternalInput").ap()
    if masked:
        wmask = nc.dram_tensor("wmask", [128, NSLOT_TOT * SW], f32,
                               kind="ExternalInput").ap()

    x2_out = nc.dram_tensor("x2_out", [128, 8, S], f32,
                            kind="ExternalOutput").ap()
    xfb_out = nc.dram_tensor("xfb_out", [128, 8, S], f32,
                             kind="ExternalOutput").ap()
    lg_out = nc.dram_tensor("lg_out", [E_MLP + E_VE, S], f32,
                            kind="ExternalOutput").ap()

    _register_consts(nc, [EPS])
    with tile.TileContext(nc) as tc, ExitStack() as est:
        wp = est.enter_context(tc.tile_pool(name="wp", bufs=1))
        ytp = est.enter_context(tc.tile_pool(name="ytp", bufs=1))

        yTp = ytp.tile([128, NH // 2, S], f32r, name="yTp")
        yTo = ytp.tile([64, NH // 2, S], f32r, name="yTo")

        with tc.tile_pool(name="ps_sc", bufs=2, space="PSUM") as ps_sc, \
             tc.tile_pool(name="ps_yv", bufs=1, space="PSUM") as ps_yv, \
             tc.tile_pool(name="ps_bc", bufs=1, space="PSUM") as ps_bc, \
             tc.tile_pool(name="attp", bufs=1) as attp, \
             tc.tile_pool(name="kvs", bufs=2) as kvs, \
             tc.tile_pool(name="ptp", bufs=9) as ptp, \
             tc.tile_pool(name="ivp", bufs=2) as ivp:
            q_t = attp.tile([65, NH, NSTRIP, SW], f32r, name="q_t")
            for hg4 in range(4):
                nc.sync.dma_start(q_t[:, 4 * hg4:4 * hg4 + 4, 0, :],
                                  q_sc[:, 4 * hg4:4 * hg4 + 4, 0, :])
            for hg4 in range(4):
                nc.sync.dma_start(q_t[:, 4 * hg4:4 * hg4 + 4, 1, :],
                                  q_sc[:, 4 * hg4:4 * hg4 + 4, 1, :])
            ones64f = attp.tile([1, 64], f32, name="ones64f")
            nc.vector.memset(ones64f[:], 1.0)
            ones64 = attp.tile([1, 64], f32r, name="ones64")
            nc.scalar.copy(ones64[:], ones64f[:])
            if masked:
                wm_t = attp.tile([128, NSLOT_TOT * SW], f32, name="wm_t")
                nc.sync.dma_start(wm_t[:], wmask[:])

            # stream kt/v per (strip, kv-pair); each slice loaded once
            kv_tiles = {}
            for strip in range(NSTRIP):
                n_s = NPOS[strip]
                for kp in range(4):
                    kt = kvs.tile([65, 2, n_s, 128], f32r,
                                  tag=f"kt{strip}", name=f"kt{strip}_{kp}")
                    vt = kvs.tile([128, 2, n_s, 65], f32r,
                                  tag=f"vt{strip}", name=f"vt{strip}_{kp}")
                    sl = slice(POS_BASE[strip], POS_BASE[strip] + n_s)
                    nc.sync.dma_start(kt[:], kt_sc[:, 2 * kp:2 * kp + 2, sl, :])
                    nc.sync.dma_start(vt[:], v_sc[:, 2 * kp:2 * kp + 2, sl, :])
                    kv_tiles[(strip, kp)] = (kt, vt)
            x_t = wp.tile([128, 8, S], f32, name="x_t")
            rw_t = wp.tile([128, 8, E_MLP + E_VE], f32, name="rw_t")
            nc.sync.dma_start(x_t[:], x_fm32[:])
            nc.sync.dma_start(rw_t[:], rw_sb[:])

            def emit_scores(strip, hg):
                """Scores + exp + mask for the 4 heads of kv-pair hg."""
                n_s = NPOS[strip]
                kt, _ = kv_tiles[(strip, hg)]
                chunks = [(c0, min(4, n_s - c0)) for c0 in range(0, n_s, 4)]
                pts = []
                for hi in range(4):
                    h = 4 * hg + hi
                    kvl = hi // 2          # kv head within the pair
                    pt_chunks = []
                    for c0, cn in chunks:
                        sc = ps_sc.tile([128, 4 * SW], f32, tag="sc",
                                        name="sc")
                        for s in range(cn):
                            nc.tensor.matmul(
                                sc[:, bass.ts(s, SW)],
                                kt[:, kvl, c0 + s, :],
                                q_t[:, h, strip, :],
                                start=True, stop=True)
                        pt = ptp.tile([128, 4 * SW], f32r, tag="pt",
                                      name=f"pt{strip}_{h}_{c0}")
                        nc.scalar.activation(pt[:, 0:cn * SW],
                                             sc[:, 0:cn * SW],
                                             AF.Exp, scale=0.125)
                        if masked:
                            base = (POS_BASE[strip] + c0) * SW
                            nc.vector.tensor_mul(
                                pt[:, 0:cn * SW], pt[:, 0:cn * SW],
                                wm_t[:, base:base + cn * SW])
                        else:
                            if c0 + cn == n_s:
                                # main diagonal tile (last slot): q-half 0
                                # is fully future -> zero; q-half 1: k <= q
                                off = (cn - 1) * SW
                                zsl = pt[:, off:off + 128]
                                nc.gpsimd.affine_select(
                                    zsl, zsl, pattern=[[1, 128]], base=-128,
                                    channel_multiplier=-1,
                                    compare_op=AluOpType.is_ge, fill=0.0)
                                dsl = pt[:, off + 128:off + 256]
                                nc.gpsimd.affine_select(
                                    dsl, dsl, pattern=[[1, 128]], base=0,
                                    channel_multiplier=-1,
                                    compare_op=AluOpType.is_ge, fill=0.0)
                                # sub-diagonal (slot n_s-2): q-half 0 k<=q
                                if cn >= 2:
                                    ssl = pt[:, off - SW:off - 128]
                                    nc.gpsimd.affine_select(
                                        ssl, ssl, pattern=[[1, 128]], base=0,
                                        channel_multiplier=-1,
                                        compare_op=AluOpType.is_ge, fill=0.0)
                        pt_chunks.append(pt)
                    pts.append(pt_chunks)
                return pts, chunks

            def emit_yv(strip, hg, pts, chunks):
                """p@v accumulate (ones col 64 -> den row 64) + normalize."""
                n_s = NPOS[strip]
                _, vt = kv_tiles[(strip, hg)]
                yv_ps = ps_yv.tile([65, 4 * SW], f32, tag="yv",
                                   name=f"yv{strip}_{hg}")
                for hi in range(4):
                    kvl = hi // 2
                    for (c0, cn), pt in zip(chunks, pts[hi]):
                        for s in range(cn):
                            nc.tensor.matmul(
                                yv_ps[:, bass.ts(hi, SW)],
                                vt[:, kvl, c0 + s, :],
                                pt[:, bass.ts(s, SW)],
                                start=(c0 + s == 0),
                                stop=(c0 + s == n_s - 1))
                iv = ivp.tile([1, 4 * SW], f32r, tag="iv", name="iv")
                with nc.allow_low_precision(reason="softmax recip"):
                    nc.vector.reciprocal(iv[:], yv_ps[64:65, :])
                bc_ps = ps_bc.tile([64, 4 * SW], f32, tag="bc", name="bc")
                for hi in range(4):
                    nc.tensor.matmul(bc_ps[:, bass.ts(hi, SW)],
                                     ones64[:], iv[0:1, bass.ts(hi, SW)],
                                     start=True, stop=True)
                bc_sb = ivp.tile([64, 4 * SW], f32, tag="bcs", name="bcs")
                nc.vector.tensor_copy(bc_sb[:], bc_ps[:])
                yv4 = yv_ps[0:64, :].rearrange("p (h n) -> p h n", h=4)
                bc4 = bc_sb[:].rearrange("p (h n) -> p h n", h=4)
                ssl = bass.ts(strip, SW)
                # even heads (hi 0,2) -> chunks 2hg..2hg+1 rows 0:64
                nc.vector.tensor_mul(
                    yTp[0:64, 2 * hg:2 * hg + 2, ssl],
                    yv4[:, 0:4:2, :], bc4[:, 0:4:2, :])
                # odd heads -> staging, then partition-shift DMA
                nc.vector.tensor_mul(
                    yTo[:, 2 * hg:2 * hg + 2, ssl],
                    yv4[:, 1:4:2, :], bc4[:, 1:4:2, :])
                nc.sync.dma_start(yTp[64:128, 2 * hg:2 * hg + 2, ssl],
                                  yTo[:, 2 * hg:2 * hg + 2, ssl])

            pending = deque()
            LAG = 2
            for strip in range(NSTRIP):
                for hg in range(4):
                    pts, chunks = emit_scores(strip, hg)
                    pending.append((strip, hg, pts, chunks))
                    if len(pending) > LAG:
                        emit_yv(*pending.popleft())
            while pending:
                emit_yv(*pending.popleft())

        # ---- wo + residual + xf rmsnorm + router ----
        with tc.tile_pool(name="ps_at", bufs=2, space="PSUM") as ps_at, \
             tc.tile_pool(name="ps_row", bufs=2, space="PSUM") as ps_row, \
             tc.tile_pool(name="ps_bcf", bufs=1, space="PSUM") as ps_bcf, \
             tc.tile_pool(name="tl", bufs=2) as tl, \
             tc.tile_pool(name="x2p", bufs=1) as x2p:
            ones_f = tl.tile([128, 1], f32, tag="onesf", name="ones_f", bufs=1)
            nc.vector.memset(ones_f[:], 1.0)
            ones_col = tl.tile([128, 1], f32r, tag="onesc", name="ones_col",
                               bufs=1)
            nc.scalar.copy(ones_col[:], ones_f[:])
            ones_rf = tl.tile([1, 128], f32, tag="onesrf", name="ones_rf",
                              bufs=1)
            nc.vector.memset(ones_rf[:], 1.0)
            ones_row = tl.tile([1, 128], f32r, tag="onesr", name="ones_row",
                               bufs=1)
            nc.scalar.copy(ones_row[:], ones_rf[:])

            x2w = x2p.tile([128, 8, S], f32, name="x2w")
            ssq_f = ps_bcf.tile([1, S], f32, tag="ssqf", name="ssq_f")
            rt_ps = ps_row.tile([E_MLP + E_VE, S], f32, tag="rt", name="rt_ps")
            wo_tiles = []
            for co in range(8):
                wo_t = tl.tile([128, 8, 128], f32r, tag="wo",
                               name=f"wo{co}", bufs=5)
                nc.sync.dma_start(wo_t[:], wo_sc[:, :, bass.ts(co, 128)])
                wo_tiles.append(wo_t)
            sqfs = []
            for co in range(8):
                at_ps = ps_at.tile([128, S], f32, tag="at", name="at_ps")
                for cc in range(8):
                    nc.tensor.matmul(
                        at_ps[:], wo_tiles[co][:, cc, :],
                        yTp[:, cc, :],
                        start=(cc == 0), stop=(cc == 7))
                nc.vector.tensor_add(x2w[:, co, :], at_ps[:], x_t[:, co, :])
                nc.sync.dma_start(x2_out[:, co, :], x2w[:, co, :])
                sqf = tl.tile([128, S], f32r, tag="sqf", name=f"sqf{co}",
                              bufs=8)
                nc.scalar.activation(sqf[:], x2w[:, co, :], AF.Square)
                sqfs.append(sqf)
            for co in range(8):
                nc.tensor.matmul(ssq_f[:], ones_col[:], sqfs[co][:],
                                 start=(co == 0), stop=(co == 7))
                nc.tensor.matmul(rt_ps[:], rw_t[:, co, :], x2w[:, co, :],
                                 start=(co == 0), stop=(co == 7))

            srow = tl.tile([1, S], f32, tag="srow", name="srow", bufs=1)
            rrow = tl.tile([1, S], f32r, tag="rrow", name="rrow", bufs=1)
            bcf_sb = tl.tile([128, S], f32, tag="bcfs", name="bcf_sb", bufs=1)
            xfb = x2p.tile([128, 8, S], f32, name="xfb")
            lg = tl.tile([E_MLP + E_VE, S], f32, tag="lg", name="lg", bufs=1)
            for hf in range(2):
                fsl = bass.ts(hf, SW)
                nc.scalar.activation(srow[0:1, fsl], ssq_f[0:1, fsl],
                                     AF.Sqrt, bias=EPS, scale=1.0 / C)
                with nc.allow_low_precision(reason="f32r rms bcast rows"):
                    nc.vector.reciprocal(rrow[0:1, fsl], srow[0:1, fsl])
                bcf_ps = ps_row.tile([128, SW], f32, tag="bcf", name="bcf_ps")
                nc.tensor.matmul(bcf_ps[:], ones_row[:], rrow[0:1, fsl],
                                 start=True, stop=True)
                nc.vector.tensor_copy(bcf_sb[:, fsl], bcf_ps[:])
                nc.vector.tensor_mul(lg[:, fsl], rt_ps[:, fsl],
                                     bcf_sb[0:E_MLP + E_VE, fsl])
                for co in range(8):
                    eng = nc.vector if co % 2 == 0 else nc.gpsimd
                    eng.tensor_mul(xfb[:, co, fsl], x2w[:, co, fsl],
                                   bcf_sb[:, fsl])
                    nc.sync.dma_start(xfb_out[:, co, fsl], xfb[:, co, fsl])
            nc.sync.dma_start(lg_out[:], lg[:])

    nc.compile()
    return nc


# --------------------------------------------------------------------------
# L2: expert MLP (bf16 up, fp8-DR down, gate pre-folded) + VE sum
# --------------------------------------------------------------------------
def build_2(ncap: int):
    nc = bacc.Bacc("TRN2", target_bir_lowering=False, debug=False,
                   num_devices=NCORES)
    NT = ncap // 512

    xfg = nc.dram_tensor("xfg", [128, 8, ncap], bf16,
                         kind="ExternalInput").ap()
    wup = nc.dram_tensor("wup", [128, 8, HID], bf16,
                         kind="ExternalInput").ap()
    wdn = nc.dram_tensor("wdn", [128, 16, C], bf16,
                         kind="ExternalInput").ap()
    ve0 = nc.dram_tensor("ve0", [128, QT, C], fp8e4, kind="ExternalInput").ap()
    ve1 = nc.dram_tensor("ve1", [128, QT, C], fp8e4, kind="ExternalInput").ap()

    moe_out = nc.dram_tensor("moe_out", [128, 8, ncap], bf16,
                             kind="ExternalOutput").ap()
    ve_out = nc.dram_tensor("ve_out", [128, QT, C], bf16,
                            kind="ExternalOutput").ap()

    with tile.TileContext(nc) as tc, ExitStack() as est:
        wp = est.enter_context(tc.tile_pool(name="wp", bufs=1))
        hp = est.enter_context(tc.tile_pool(name="hp", bufs=2))
        op = est.enter_context(tc.tile_pool(name="op", bufs=3))
        ps_h = est.enter_context(tc.tile_pool(name="ps_h", bufs=3, space="PSUM"))
        ps_o = est.enter_context(tc.tile_pool(name="ps_o", bufs=3, space="PSUM"))

        xf_t = wp.tile([128, 8, ncap], bf16, name="xf_t")
        up_t = wp.tile([128, 8, HID], bf16, name="up_t")
        dn_t = wp.tile([128, 16, C], bf16, name="dn_t")
        v0_t = wp.tile([128, QT, C], fp8e4, name="v0_t")
        v1_t = wp.tile([128, QT, C], fp8e4, name="v1_t")
        nc.sync.dma_start(xf_t[:], xfg[:])
        for cc in range(8):
            nc.sync.dma_start(up_t[:, cc, :], wup[:, cc, :])
        for cc in range(8):
            nc.sync.dma_start(dn_t[:, 2 * cc:2 * cc + 2, :],
                              wdn[:, 2 * cc:2 * cc + 2, :])
        nc.sync.dma_start(v0_t[:], ve0[:])
        nc.sync.dma_start(v1_t[:], ve1[:])

        for nt in range(NT):
            csl = bass.ts(nt, 512)
            h_sb = hp.tile([128, 16, 512], bf16, tag="h", name=f"h{nt}")
            for hc in range(16):
                h_ps = ps_h.tile([128, 512], f32, tag="hps", name="h_ps")
                for cc in range(8):
                    nc.tensor.matmul(
                        h_ps[:], up_t[:, cc, bass.ts(hc, 128)],
                        xf_t[:, cc, csl],
                        start=(cc == 0), stop=(cc == 7))
                hr = op.tile([128, 512], bf16, tag="hr", name="hr")
                nc.scalar.activation(hr[:], h_ps[:], AF.Relu)
                eng = nc.vector if hc % 3 != 2 else nc.gpsimd
                eng.tensor_mul(h_sb[:, hc, :], hr[:], hr[:])
            for co in range(8):
                o_ps = ps_o.tile([128, 512], f32, tag="ops", name="o_ps")
                for hh in range(16):
                    nc.tensor.matmul(
                        o_ps[:], dn_t[:, hh, bass.ts(co, 128)],
                        h_sb[:, hh, :],
                        start=(hh == 0), stop=(hh == 15))
                ot = op.tile([128, 512], bf16, tag="ot", name="ot")
                nc.scalar.copy(ot[:], o_ps[:])
                nc.sync.dma_start(moe_out[:, co, csl], ot[:])

        vo = op.tile([128, QT, C], bf16, tag="vo", name="vo", bufs=1)
        for t in range(QT):
            eng = nc.vector if t % 2 == 0 else nc.gpsimd
            eng.tensor_add(vo[:, t, :], v0_t[:, t, :], v1_t[:, t, :])
        nc.sync.dma_start(ve_out[:], vo[:])

    nc.compile()
    return nc


# --------------------------------------------------------------------------
# Host orchestration
# --------------------------------------------------------------------------
def _qtiles(ci):
    # strip A = 256-tile ci (128-tiles 2ci, 2ci+1), strip B = 256-tile 7-ci
    sa, sb = ci, 7 - ci
    return [2 * sa, 2 * sa + 1, 2 * sb, 2 * sb + 1]


def _slot_map(st, n_s, window):
    """slot -> (key 128-tile, bias).  Diagonal tile (2st+1) at slot n_s-1,
    sub-diagonal (2st) at n_s-2, other causally-alive tiles from slot 0."""
    alive = [kt for kt in range(2 * st)
             if window >= T or 128 * (2 * st + 1 - kt) - 127 <= window]
    m = {}
    for si, kt in enumerate(alive):
        m[si] = (kt, 0.0)
    m[n_s - 2] = (2 * st, 0.0)
    m[n_s - 1] = (2 * st + 1, 0.0)
    return m


def _route(logits, router_bias):
    sig = (1.0 / (1.0 + np.exp(-logits.astype(np.float32)))).astype(np.float32)
    sel = sig + router_bias[None, :].astype(np.float32)
    idx = np.argsort(-sel, axis=1, kind="stable")[:, :TOPK]
    tw = np.take_along_axis(sig, idx, axis=1)
    tw = tw / tw.sum(axis=1, keepdims=True)
    N = logits.shape[0]
    sparse_w = np.zeros((N, E_MLP + E_VE), np.float32)
    np.put_along_axis(sparse_w, idx, tw, axis=1)
    return sparse_w


def kernel(**inputs):
    x = np.asarray(inputs["x"], np.float32)
    token_ids = np.asarray(inputs["token_ids"])
    cos = np.asarray(inputs["cos"], np.float32)
    sin = np.asarray(inputs["sin"], np.float32)
    window = int(np.asarray(inputs["window_size"]))
    wq, wk, wv, wo = (np.asarray(inputs[k], np.float32)
                      for k in ("wq", "wk", "wv", "wo"))
    w_up = np.asarray(inputs["w_up"], np.float32)
    w_down = np.asarray(inputs["w_down"], np.float32)
    router_w = np.asarray(inputs["router_w"], np.float32)
    router_bias = np.asarray(inputs["router_bias"], np.float32)
    ve_tables = np.asarray(inputs["ve_tables"], np.float32)

    cosf = cos[0, :, 0, :]      # (T, 32)
    sinf = sin[0, :, 0, :]

    # ---------------- L1a ----------------
    if "1a" not in _prog_cache:
        _prog_cache["1a"] = build_1a()
    nc1a = _prog_cache["1a"]

    def mov_w(w):  # (M, C) -> [128, 8, M] with [p, cc, m] = w[m, 128cc+p]
        M = w.shape[0]
        return np.ascontiguousarray(
            w.T.reshape(8, 128, M).transpose(1, 0, 2)).astype(np.float32)

    wq_t, wk_t, wv_t = mov_w(wq), mov_w(wk), mov_w(wv)
    toks_all, maps1a = [], []
    for c in range(NCORES):
        b, ci = c // 4, c % 4
        qts = _qtiles(ci)
        toks = np.concatenate([np.arange(qt * 128, qt * 128 + 128)
                               for qt in qts])
        toks_all.append(toks)
        xs = x[b, toks, :]                      # (512, 1024)
        x_fm = np.ascontiguousarray(
            xs.T.reshape(8, 128, S).transpose(1, 0, 2)).astype(np.float32)
        x_tm = np.ascontiguousarray(
            xs.reshape(QT, 128, C).transpose(1, 0, 2)).astype(np.float32)
        cos_tm = np.ascontiguousarray(
            cosf[toks].reshape(QT, 128, 32).transpose(1, 0, 2)
        ).astype(np.float32)
        sin_tm = np.ascontiguousarray(
            sinf[toks].reshape(QT, 128, 32).transpose(1, 0, 2)
        ).astype(np.float32)
        maps1a.append(dict(x_fm=x_fm, x_tm=x_tm, wq_t=wq_t, wk_t=wk_t,
                           wv_t=wv_t, cos_tm=cos_tm, sin_tm=sin_tm))
    res1a = run_bass_kernel_spmd(nc1a, maps1a, list(range(NCORES))).results

    kn = np.zeros((B, T, NKV * HD), np.float32)
    vn = np.zeros((B, T, NKV * HD), np.float32)
    qn = []
    for c in range(NCORES):
        b = c // 4
        toks = toks_all[c]
        kc = res1a[c]["k_out"].astype(np.float32)   # [128, QT, 512]
        vc = res1a[c]["v_out"].astype(np.float32)
        kn[b, toks.reshape(QT, 128)] = kc.transpose(1, 0, 2)
        vn[b, toks.reshape(QT, 128)] = vc.transpose(1, 0, 2)
        qn.append(res1a[c]["q_out"].astype(np.float32))  # [128, QT, 1024]

    # ---------------- L1b ----------------
    masked = window < T
    key1b = ("1b", masked)
    if key1b not in _prog_cache:
        _prog_cache[key1b] = build_1b(masked)
    nc1b = _prog_cache[key1b]

    wo_sc = np.ascontiguousarray(
        wo.T.reshape(8, 128, C).transpose(1, 0, 2)).astype(np.float32)
    rw_sb = np.ascontiguousarray(
        router_w.T.reshape(8, 128, E_MLP + E_VE).transpose(1, 0, 2)
    ).astype(np.float32)

    maps1b = []
    for c in range(NCORES):
        b, ci = c // 4, c % 4
        strips = [ci, 7 - ci]          # 256-token strip indices
        toks = toks_all[c]
        q_sc = np.zeros((65, NH, NSTRIP, SW), np.float32)
        qc = qn[c]                     # [128, QT, 1024] token-major
        qtok = qc.transpose(1, 0, 2).reshape(S, NH, HD)   # (512, NH, 64)
        q_sc[0:64] = qtok.reshape(NSTRIP, SW, NH, HD).transpose(3, 2, 0, 1)
        q_sc[64] = 1.0
        kt_sc = np.zeros((65, NKV, NSLOT_TOT, 128), np.float32)
        kt_sc[64] = DEAD_BIAS
        v_sc = np.zeros((128, NKV, NSLOT_TOT, 65), np.float32)
        v_sc[:, :, :, 64] = 1.0
        kb = kn[b].reshape(16, 128, NKV, HD)   # [ktile, n, kv, d]
        vb = vn[b].reshape(16, 128, NKV, HD)
        for strip in range(NSTRIP):
            st = strips[strip]
            n_s = NPOS[strip]
            smap = _slot_map(st, n_s, window)
            for s, (kt, bias) in smap.items():
                po = POS_BASE[strip] + s
                kt_sc[0:64, :, po, :] = kb[kt].transpose(2, 1, 0)
                kt_sc[64, :, po, :] = bias
                v_sc[:, :, po, 0:64] = vb[kt]
        xs = x[b, toks, :]
        x_fm32 = np.ascontiguousarray(
            xs.T.reshape(8, 128, S).transpose(1, 0, 2)).astype(np.float32)
        m = dict(q_sc=q_sc, kt_sc=np.ascontiguousarray(kt_sc),
                 v_sc=np.ascontiguousarray(v_sc), x_fm32=x_fm32,
                 wo_sc=wo_sc, rw_sb=rw_sb)
        if masked:
            wm = np.zeros((128, NSLOT_TOT * SW), np.float32)
            for strip in range(NSTRIP):
                st = strips[strip]
                n_s = NPOS[strip]
                smap = _slot_map(st, n_s, window)
                for s, (kt, bias) in smap.items():
                    po = POS_BASE[strip] + s
                    qg = 2 * st * 128 + np.arange(SW)[None, :]
                    kg = kt * 128 + np.arange(128)[:, None]
                    ok = (kg <= qg) & (qg - kg <= window)
                    wm[:, po * SW:(po + 1) * SW] = ok
            m["wmask"] = wm.astype(np.float32)
        maps1b.append(m)
    res1b = run_bass_kernel_spmd(nc1b, maps1b, list(range(NCORES))).results

    # ---------------- routing ----------------
    N = B * T
    logits = np.zeros((N, E_MLP + E_VE), np.float32)
    x2 = np.zeros((N, C), np.float32)
    xfb = np.zeros((N, C), np.float32)
    for c in range(NCORES):
        b = c // 4
        toks = toks_all[c] + b * T
        logits[toks] = res1b[c]["lg_out"].T
        x2c = res1b[c]["x2_out"]                     # [128 p, 8 cc, 512 n]
        x2[toks] = x2c.transpose(2, 1, 0).reshape(S, C)
        xfc = res1b[c]["xfb_out"].astype(np.float32)
        xfb[toks] = xfc.transpose(2, 1, 0).reshape(S, C)

    sparse_w = _route(logits, router_bias)

    idx_list = [np.nonzero(sparse_w[:, e])[0] for e in range(E_MLP)]
    n_list = [len(ix) for ix in idx_list]
    ncap = NCAP0
    while ncap < max(n_list):
        ncap += 512

    key2 = ("2", ncap)
    if key2 not in _prog_cache:
        _prog_cache[key2] = build_2(ncap)
    nc2 = _prog_cache[key2]

    tok_flat = token_ids.reshape(-1)
    maps2 = []
    for c in range(NCORES):
        e = c
        ix = idx_list[e]
        g = np.sqrt(sparse_w[ix, e]).astype(np.float32)
        xg = np.zeros((C, ncap), np.float32)
        xg[:, :len(ix)] = xfb[ix].T * g[None, :]
        xfg = np.ascontiguousarray(
            xg.reshape(8, 128, ncap).transpose(1, 0, 2)).astype(BF)
        wup_m = np.ascontiguousarray(
            w_up[e].T.reshape(8, 128, HID).transpose(1, 0, 2)).astype(BF)
        wdn_dr = np.ascontiguousarray(
            w_down[e].T.reshape(16, 128, C).transpose(1, 0, 2)).astype(BF)
        b = c // 4
        toks = toks_all[c] + b * T
        tids = tok_flat[toks]
        ve0 = (64.0 * sparse_w[toks, E_MLP, None]
               * ve_tables[0][tids]).reshape(QT, 128, C).transpose(1, 0, 2)
        ve1 = (64.0 * sparse_w[toks, E_MLP + 1, None]
               * ve_tables[1][tids]).reshape(QT, 128, C).transpose(1, 0, 2)
        maps2.append(dict(xfg=xfg, wup=wup_m, wdn=wdn_dr,
                          ve0=np.ascontiguousarray(ve0).astype(E4),
                          ve1=np.ascontiguousarray(ve1).astype(E4)))
    res2 = run_bass_kernel_spmd(nc2, maps2, list(range(NCORES))).results

    out = x2.copy()
    for c in range(NCORES):
        b = c // 4
        toks = toks_all[c] + b * T
        veo = res2[c]["ve_out"].astype(np.float32)  # [128, QT, C]
        out[toks] += veo.transpose(1, 0, 2).reshape(S, C) / 64.0
    for e in range(E_MLP):
        n_e = n_list[e]
        if n_e:
            moe = res2[e]["moe_out"].astype(np.float32)  # [128, 8, ncap]
            out[idx_list[e]] += moe[:, :, :n_e].transpose(
                2, 1, 0).reshape(n_e, C)
    return out.reshape(B, T, C).astype(np.float32)


# revision 47
# speedup vs baseline: 1.0101x; 1.0101x over previous
"""TRN2 Bass kernel for nn_BlockMoVaE (attention + MoE/VE routing block).

Self-contained: accepts FULL inputs, shards across 8 NeuronCores, returns
FULL output.

Three SPMD launches with host re-layout between them (host does only data
movement / routing; all FLOPs stay on device).  The router top-2 decision
is discrete, so everything feeding the logits (all of phase 1) runs at
f32r precision; only the post-routing expert MLP uses bf16/fp8.

  L1a  (token-parallel, 512 tokens/core = 2 causally-balanced 256-strips):
       x-rms stats, raw-x f32r QKV projections, token-major rope +
       per-head rmsnorm (the per-token x-norm scalar cancels inside the
       head rmsnorm, so Q/K project raw x; V is scaled by r explicitly).
       Exports token-major q/k/v (f32).
  L1b  (token-parallel attention): host re-lays q/k/v into score-friendly
       f32r layouts.  Causal work is balanced by giving core ci the
       256-token strips {ci, 7-ci} of its batch; the static program
       computes (8, 16) key-128-slots for the two strips; fully-dead
       slots are killed by a rank-1 bias row (-30000) folded into the
       score matmul contraction, diagonal tiles by static affine_selects.
       Softmax denominators ride along as a ones-column of V.  Then
       wo + residual + xf rmsnorm + router logits.
  L2   (expert-parallel MoE): host routes top-2 and gathers tokens per
       expert with sqrt(gate) pre-scaling (relu^2 is 2-homogeneous so the
       gate factors exactly); bf16 up / fp8-DoubleRow down projections;
       VE rows host-gathered, gate-pre-scaled, summed on device.
"""
import numpy as np
import ml_dtypes

import concourse.bass as bass
import concourse.bacc as bacc
import concourse.mybir as mybir
import concourse.tile as tile
from concourse.bass_utils import run_bass_kernel_spmd
from concourse.alu_op_type import AluOpType
from contextlib import ExitStack
from collections import deque

# ---- problem constants (hardcoded per contest rules) ----
B, T, C = 2, 2048, 1024
NH, NKV, HD = 16, 8, 64
E_MLP, E_VE, TOPK = 8, 2, 2
HID = 2048
VOCAB = 50257
EPS = 1e-6
NCORES = 8
S = 512              # tokens per core
QT = 4               # 128-token tiles per core (2 strips of 256)
NSTRIP = 2
SW = 256             # strip width (queries)
NPOS = (8, 16)       # static key-slot count per strip
POS_BASE = (0, 8)    # slot base in kT layout (total 24)
NSLOT_TOT = 24
DEAD_BIAS = -30000.0
NCAP0 = 1024

f32 = mybir.dt.float32
f32r = mybir.dt.float32r
bf16 = mybir.dt.bfloat16
fp8e4 = mybir.dt.float8e4
AF = mybir.ActivationFunctionType
DR = mybir.MatmulPerfMode.DoubleRow
E4 = ml_dtypes.float8_e4m3
BF = ml_dtypes.bfloat16

_prog_cache = {}


def _register_consts(nc, values):
    for value in values:
        key = (f32, float(value))
        if key not in nc.const_aps.aps:
            t = nc.alloc_sbuf_tensor(f"constap-{value}", [128, 1], f32)
            nc.gpsimd.memset(t.ap(), float(value))
            nc.const_aps.aps[key] = t.ap()
    nc.all_engine_barrier()


# --------------------------------------------------------------------------
# L1a: x stats + QKV projection + rope + head-rms (token-major epilogues)
# --------------------------------------------------------------------------
def build_1a():
    nc = bacc.Bacc("TRN2", target_bir_lowering=False, debug=False,
                   num_devices=NCORES)

    x_fm = nc.dram_tensor("x_fm", [128, 8, S], f32r, kind="ExternalInput").ap()
    x_tm = nc.dram_tensor("x_tm", [128, QT, C], f32, kind="ExternalInput").ap()
    wq_t = nc.dram_tensor("wq_t", [128, 8, NH * HD], f32r,
                          kind="ExternalInput").ap()
    wk_t = nc.dram_tensor("wk_t", [128, 8, NKV * HD], f32r,
                          kind="ExternalInput").ap()
    wv_t = nc.dram_tensor("wv_t", [128, 8, NKV * HD], f32r,
                          kind="ExternalInput").ap()
    cos_tm = nc.dram_tensor("cos_tm", [128, QT, 32], f32,
                            kind="ExternalInput").ap()
    sin_tm = nc.dram_tensor("sin_tm", [128, QT, 32], f32,
                            kind="ExternalInput").ap()

    q_out = nc.dram_tensor("q_out", [128, QT, NH * HD], f32,
                           kind="ExternalOutput").ap()
    k_out = nc.dram_tensor("k_out", [128, QT, NKV * HD], f32,
                           kind="ExternalOutput").ap()
    v_out = nc.dram_tensor("v_out", [128, QT, NKV * HD], f32,
                           kind="ExternalOutput").ap()

    _register_consts(nc, [EPS])
    with tile.TileContext(nc) as tc, ExitStack() as est:
        wp = est.enter_context(tc.tile_pool(name="wp", bufs=1))
        work = est.enter_context(tc.tile_pool(name="work", bufs=2))
        outp = est.enter_context(tc.tile_pool(name="outp", bufs=1))
        ps_q = est.enter_context(tc.tile_pool(name="ps_q", bufs=2, space="PSUM"))
        ps_k = est.enter_context(tc.tile_pool(name="ps_k", bufs=2, space="PSUM"))
        ps_v = est.enter_context(tc.tile_pool(name="ps_v", bufs=2, space="PSUM"))

        xf = wp.tile([128, 8, S], f32r, name="xf")
        xt = wp.tile([128, QT, C], f32, name="xt")
        wq = wp.tile([128, 8, NH * HD], f32r, name="wq")
        wk = wp.tile([128, 8, NKV * HD], f32r, name="wk")
        wv = wp.tile([128, 8, NKV * HD], f32r, name="wv")
        cs = wp.tile([128, QT, 32], f32, name="cs")
        sn = wp.tile([128, QT, 32], f32, name="sn")
        for cc in range(8):
            nc.sync.dma_start(xf[:, cc, :], x_fm[:, cc, :])
            nc.sync.dma_start(wq[:, cc, :], wq_t[:, cc, :])
        for cc in range(8):
            nc.sync.dma_start(wk[:, cc, :], wk_t[:, cc, :])
            nc.sync.dma_start(wv[:, cc, :], wv_t[:, cc, :])
        nc.sync.dma_start(cs[:], cos_tm[:])
        nc.sync.dma_start(sn[:], sin_tm[:])
        nc.sync.dma_start(xt[:], x_tm[:])

        qe = outp.tile([128, QT, NH * HD], f32, name="qe")
        ke = outp.tile([128, QT, NKV * HD], f32, name="ke")
        ve = outp.tile([128, QT, NKV * HD], f32, name="ve")

        def rope_norm(ps, nh, t, out_tile):
            """Token-major rope + per-head rmsnorm from psum [128, nh*64]."""
            qs = work.tile([128, nh, HD], f32, tag=f"qs{nh}", name="qs")
            nc.scalar.copy(qs[:], ps[:].rearrange("p (h d) -> p h d", d=HD))
            cosb = cs[:, t:t + 1, :].broadcast_to([128, nh, 32])
            sinb = sn[:, t:t + 1, :].broadcast_to([128, nh, 32])
            rp = work.tile([128, nh, HD], f32, tag=f"rp{nh}", name="rp")
            a = work.tile([128, nh, 32], f32, tag=f"ra{nh}", name="ra")
            b = work.tile([128, nh, 32], f32, tag=f"rb{nh}", name="rb")
            c2 = work.tile([128, nh, 32], f32, tag=f"rc{nh}", name="rc")
            d2 = work.tile([128, nh, 32], f32, tag=f"rd{nh}", name="rd")
            nc.vector.tensor_mul(a[:], qs[:, :, 0:32], cosb)
            nc.vector.tensor_mul(b[:], qs[:, :, 32:64], sinb)
            nc.gpsimd.tensor_mul(c2[:], qs[:, :, 32:64], cosb)
            nc.gpsimd.tensor_mul(d2[:], qs[:, :, 0:32], sinb)
            nc.gpsimd.tensor_add(rp[:, :, 0:32], a[:], b[:])
            nc.vector.tensor_sub(rp[:, :, 32:64], c2[:], d2[:])
            sq = work.tile([128, nh, HD], f32, tag=f"sq{nh}", name="sq")
            nc.scalar.activation(sq[:], rp[:], AF.Square)
            hs = work.tile([128, nh, 1], f32, tag=f"hs{nh}", name="hs")
            nc.vector.tensor_reduce(out=hs[:], in_=sq[:], op=AluOpType.add,
                                    axis=mybir.AxisListType.X)
            sh = work.tile([128, nh, 1], f32, tag=f"sh{nh}", name="sh")
            nc.scalar.activation(sh[:], hs[:], AF.Sqrt, bias=EPS,
                                 scale=1.0 / HD)
            rh = work.tile([128, nh, 1], f32, tag=f"rh{nh}", name="rh")
            with nc.allow_low_precision(reason="head rms recip"):
                nc.vector.reciprocal(rh[:], sh[:])
            nc.vector.tensor_mul(
                out_tile[:].rearrange("p (h d) -> p h d", d=HD),
                rp[:], rh[:].broadcast_to([128, nh, HD]))

        for t in range(QT):
            # per-token inv-rms of x (V only; cancels inside Q/K head-rms)
            xsq = work.tile([128, C], f32, tag="xsq", name="xsq")
            nc.scalar.activation(xsq[:], xt[:, t, :], AF.Square)
            ssq = work.tile([128, 1], f32, tag="ssq", name="ssq")
            nc.vector.tensor_reduce(out=ssq[:], in_=xsq[:], op=AluOpType.add,
                                    axis=mybir.AxisListType.XYZW)
            sx = work.tile([128, 1], f32, tag="sx", name="sx")
            nc.scalar.activation(sx[:], ssq[:], AF.Sqrt, bias=EPS,
                                 scale=1.0 / C)
            r = work.tile([128, 1], f32, tag="r", name="r")
            with nc.allow_low_precision(reason="x rms recip"):
                nc.vector.reciprocal(r[:], sx[:])

            q_ps = ps_q.tile([128, NH * HD], f32, name="q_ps")
            k_ps = ps_k.tile([128, NKV * HD], f32, name="k_ps")
            v_ps = ps_v.tile([128, NKV * HD], f32, name="v_ps")
            for half in range(2):
                hsl = bass.ts(half, NH * HD // 2)
                for cc in range(8):
                    nc.tensor.matmul(q_ps[:, hsl],
                                     xf[:, cc, bass.ts(t, 128)],
                                     wq[:, cc, hsl],
                                     start=(cc == 0), stop=(cc == 7))
            for cc in range(8):
                nc.tensor.matmul(k_ps[:], xf[:, cc, bass.ts(t, 128)],
                                 wk[:, cc, :], start=(cc == 0), stop=(cc == 7))
            for cc in range(8):
                nc.tensor.matmul(v_ps[:], xf[:, cc, bass.ts(t, 128)],
                                 wv[:, cc, :], start=(cc == 0), stop=(cc == 7))

            rope_norm(q_ps, NH, t, qe[:, t, :])
            nc.sync.dma_start(q_out[:, t, :], qe[:, t, :])
            rope_norm(k_ps, NKV, t, ke[:, t, :])
            nc.sync.dma_start(k_out[:, t, :], ke[:, t, :])
            nc.vector.tensor_scalar_mul(ve[:, t, :], v_ps[:], r[:])
            nc.sync.dma_start(v_out[:, t, :], ve[:, t, :])

    nc.compile()
    return nc


# --------------------------------------------------------------------------
# L1b: attention + wo + residual + xf rmsnorm + router logits
# --------------------------------------------------------------------------
def build_1b(masked: bool):
    nc = bacc.Bacc("TRN2", target_bir_lowering=False, debug=False,
                   num_devices=NCORES)

    # q rows 0..63 = head dims, row 64 = 1.0 (rank-1 bias carrier)
    q_sc = nc.dram_tensor("q_sc", [65, NH, NSTRIP, SW], f32r,
                          kind="ExternalInput").ap()
    kt_sc = nc.dram_tensor("kt_sc", [65, NKV, NSLOT_TOT, 128], f32r,
                           kind="ExternalInput").ap()
    # v columns 0..63 = v dims, col 64 = 1.0 (softmax denominator)
    v_sc = nc.dram_tensor("v_sc", [128, NKV, NSLOT_TOT, 65], f32r,
                          kind="ExternalInput").ap()
    x_fm32 = nc.dram_tensor("x_fm32", [128, 8, S], f32,
                            kind="ExternalInput").ap()
    wo_sc = nc.dram_tensor("wo_sc", [128, 8, C], f32r,
                           kind="ExternalInput").ap()
    rw_sb = nc.dram_tensor("rw_sb", [128, 8, E_MLP + E_VE], f32,
                           kind="ExternalInput").ap()
    if masked:
        wmask = nc.dram_tensor("wmask", [128, NSLOT_TOT * SW], f32,
                               kind="ExternalInput").ap()

    x2_out = nc.dram_tensor("x2_out", [128, 8, S], f32,
                            kind="ExternalOutput").ap()
    xfb_out = nc.dram_tensor("xfb_out", [128, 8, S], f32,
                             kind="ExternalOutput").ap()
    lg_out = nc.dram_tensor("lg_out", [E_MLP + E_VE, S], f32,
                            kind="ExternalOutput").ap()

    _register_consts(nc, [EPS])
    with tile.TileContext(nc) as tc, ExitStack() as est:
        wp = est.enter_context(tc.tile_pool(name="wp", bufs=1))
        ytp = est.enter_context(tc.tile_pool(name="ytp", bufs=1))

        yTp = ytp.tile([128, NH // 2, S], f32r, name="yTp")
        yTo = ytp.tile([64, NH // 2, S], f32r, name="yTo")

        with tc.tile_pool(name="ps_sc", bufs=2, space="PSUM") as ps_sc, \
             tc.tile_pool(name="ps_yv", bufs=1, space="PSUM") as ps_yv, \
             tc.tile_pool(name="ps_bc", bufs=1, space="PSUM") as ps_bc, \
             tc.tile_pool(name="attp", bufs=1) as attp, \
             tc.tile_pool(name="kvs", bufs=2) as kvs, \
             tc.tile_pool(name="ptp", bufs=9) as ptp, \
             tc.tile_pool(name="ivp", bufs=2) as ivp:
            q_t = attp.tile([65, NH, NSTRIP, SW], f32r, name="q_t")
            for hg4 in range(4):
                nc.sync.dma_start(q_t[:, 4 * hg4:4 * hg4 + 4, 0, :],
                                  q_sc[:, 4 * hg4:4 * hg4 + 4, 0, :])
            for hg4 in range(4):
                nc.sync.dma_start(q_t[:, 4 * hg4:4 * hg4 + 4, 1, :],
                                  q_sc[:, 4 * hg4:4 * hg4 + 4, 1, :])
            ones64f = attp.tile([1, 64], f32, name="ones64f")
            nc.vector.memset(ones64f[:], 1.0)
            ones64 = attp.tile([1, 64], f32r, name="ones64")
            nc.scalar.copy(ones64[:], ones64f[:])
            if masked:
                wm_t = attp.tile([128, NSLOT_TOT * SW], f32, name="wm_t")
                nc.sync.dma_start(wm_t[:], wmask[:])

            # stream kt/v per (strip, kv-pair); each slice loaded once
            kv_tiles = {}
            for strip in range(NSTRIP):
                n_s = NPOS[strip]
                for kp in range(4):
                    kt = kvs.tile([65, 2, n_s, 128], f32r,
                                  tag=f"kt{strip}", name=f"kt{strip}_{kp}")
                    vt = kvs.tile([128, 2, n_s, 65], f32r,
                                  tag=f"vt{strip}", name=f"vt{strip}_{kp}")
                    sl = slice(POS_BASE[strip], POS_BASE[strip] + n_s)
                    nc.sync.dma_start(kt[:], kt_sc[:, 2 * kp:2 * kp + 2, sl, :])
                    nc.sync.dma_start(vt[:], v_sc[:, 2 * kp:2 * kp + 2, sl, :])
                    kv_tiles[(strip, kp)] = (kt, vt)
            x_t = wp.tile([128, 8, S], f32, name="x_t")
            rw_t = wp.tile([128, 8, E_MLP + E_VE], f32, name="rw_t")
            nc.sync.dma_start(x_t[:], x_fm32[:])
            nc.sync.dma_start(rw_t[:], rw_sb[:])

            def emit_scores(strip, hg):
                """Scores + exp + mask for the 4 heads of kv-pair hg."""
                n_s = NPOS[strip]
                kt, _ = kv_tiles[(strip, hg)]
                chunks = [(c0, min(4, n_s - c0)) for c0 in range(0, n_s, 4)]
                pts = []
                for hi in range(4):
                    h = 4 * hg + hi
                    kvl = hi // 2          # kv head within the pair
                    pt_chunks = []
                    for c0, cn in chunks:
                        sc = ps_sc.tile([128, 4 * SW], f32, tag="sc",
                                        name="sc")
                        for s in range(cn):
                            nc.tensor.matmul(
                                sc[:, bass.ts(s, SW)],
                                kt[:, kvl, c0 + s, :],
                                q_t[:, h, strip, :],
                                start=True, stop=True)
                        pt = ptp.tile([128, 4 * SW], f32r, tag="pt",
                                      name=f"pt{strip}_{h}_{c0}")
                        nc.scalar.activation(pt[:, 0:cn * SW],
                                             sc[:, 0:cn * SW],
                                             AF.Exp, scale=0.125)
                        if masked:
                            base = (POS_BASE[strip] + c0) * SW
                            nc.vector.tensor_mul(
                                pt[:, 0:cn * SW], pt[:, 0:cn * SW],
                                wm_t[:, base:base + cn * SW])
                        else:
                            if c0 + cn == n_s:
                                # main diagonal tile (last slot): q-half 0
                                # is fully future -> zero; q-half 1: k <= q
                                off = (cn - 1) * SW
                                zsl = pt[:, off:off + 128]
                                nc.gpsimd.affine_select(
                                    zsl, zsl, pattern=[[1, 128]], base=-128,
                                    channel_multiplier=-1,
                                    compare_op=AluOpType.is_ge, fill=0.0)
                                dsl = pt[:, off + 128:off + 256]
                                nc.gpsimd.affine_select(
                                    dsl, dsl, pattern=[[1, 128]], base=0,
                                    channel_multiplier=-1,
                                    compare_op=AluOpType.is_ge, fill=0.0)
                                # sub-diagonal (slot n_s-2): q-half 0 k<=q
                                if cn >= 2:
                                    ssl = pt[:, off - SW:off - 128]
                                    nc.gpsimd.affine_select(
                                        ssl, ssl, pattern=[[1, 128]], base=0,
                                        channel_multiplier=-1,
                                        compare_op=AluOpType.is_ge, fill=0.0)
                        pt_chunks.append(pt)
                    pts.append(pt_chunks)
                return pts, chunks

            def emit_yv(strip, hg, pts, chunks):
                """p@v accumulate (ones col 64 -> den row 64) + normalize."""
                n_s = NPOS[strip]
                _, vt = kv_tiles[(strip, hg)]
                yv_ps = ps_yv.tile([65, 4 * SW], f32, tag="yv",
                                   name=f"yv{strip}_{hg}")
                for hi in range(4):
                    kvl = hi // 2
                    for (c0, cn), pt in zip(chunks, pts[hi]):
                        for s in range(cn):
                            nc.tensor.matmul(
                                yv_ps[:, bass.ts(hi, SW)],
                                vt[:, kvl, c0 + s, :],
                                pt[:, bass.ts(s, SW)],
                                start=(c0 + s == 0),
                                stop=(c0 + s == n_s - 1))
                iv = ivp.tile([1, 4 * SW], f32r, tag="iv", name="iv")
                with nc.allow_low_precision(reason="softmax recip"):
                    nc.vector.reciprocal(iv[:], yv_ps[64:65, :])
                bc_ps = ps_bc.tile([64, 4 * SW], f32, tag="bc", name="bc")
                for hi in range(4):
                    nc.tensor.matmul(bc_ps[:, bass.ts(hi, SW)],
                                     ones64[:], iv[0:1, bass.ts(hi, SW)],
                                     start=True, stop=True)
                bc_sb = ivp.tile([64, 4 * SW], f32, tag="bcs", name="bcs")
                nc.vector.tensor_copy(bc_sb[:], bc_ps[:])
                yv4 = yv_ps[0:64, :].rearrange("p (h n) -> p h n", h=4)
                bc4 = bc_sb[:].rearrange("p (h n) -> p h n", h=4)
                ssl = bass.ts(strip, SW)
                # even heads (hi 0,2) -> chunks 2hg..2hg+1 rows 0:64
                nc.vector.tensor_mul(
                    yTp[0:64, 2 * hg:2 * hg + 2, ssl],
                    yv4[:, 0:4:2, :], bc4[:, 0:4:2, :])
                # odd heads -> staging, then partition-shift DMA
                nc.vector.tensor_mul(
                    yTo[:, 2 * hg:2 * hg + 2, ssl],
                    yv4[:, 1:4:2, :], bc4[:, 1:4:2, :])
                nc.sync.dma_start(yTp[64:128, 2 * hg:2 * hg + 2, ssl],
                                  yTo[:, 2 * hg:2 * hg + 2, ssl])

            pending = deque()
            LAG = 2
            for strip in range(NSTRIP):
                for hg in range(4):
                    pts, chunks = emit_scores(strip, hg)
                    pending.append((strip, hg, pts, chunks))
                    if len(pending) > LAG:
                        emit_yv(*pending.popleft())
            while pending:
                emit_yv(*pending.popleft())

        # ---- wo + residual + xf rmsnorm + router ----
        with tc.tile_pool(name="ps_at", bufs=2, space="PSUM") as ps_at, \
             tc.tile_pool(name="ps_row", bufs=2, space="PSUM") as ps_row, \
             tc.tile_pool(name="ps_bcf", bufs=1, space="PSUM") as ps_bcf, \
             tc.tile_pool(name="tl", bufs=2) as tl, \
             tc.tile_pool(name="x2p", bufs=1) as x2p:
            ones_f = tl.tile([128, 1], f32, tag="onesf", name="ones_f", bufs=1)
            nc.vector.memset(ones_f[:], 1.0)
            ones_col = tl.tile([128, 1], f32r, tag="onesc", name="ones_col",
                               bufs=1)
            nc.scalar.copy(ones_col[:], ones_f[:])
            ones_rf = tl.tile([1, 128], f32, tag="onesrf", name="ones_rf",
                              bufs=1)
            nc.vector.memset(ones_rf[:], 1.0)
            ones_row = tl.tile([1, 128], f32r, tag="onesr", name="ones_row",
                               bufs=1)
            nc.scalar.copy(ones_row[:], ones_rf[:])

            x2w = x2p.tile([128, 8, S], f32, name="x2w")
            ssq_f = ps_bcf.tile([1, S], f32, tag="ssqf", name="ssq_f")
            rt_ps = ps_row.tile([E_MLP + E_VE, S], f32, tag="rt", name="rt_ps")
            wo_tiles = []
            for co in range(8):
                wo_t = tl.tile([128, 8, 128], f32r, tag="wo",
                               name=f"wo{co}", bufs=5)
                nc.sync.dma_start(wo_t[:], wo_sc[:, :, bass.ts(co, 128)])
                wo_tiles.append(wo_t)
            sqfs = []
            for co in range(8):
                at_ps = ps_at.tile([128, S], f32, tag="at", name="at_ps")
                for cc in range(8):
                    nc.tensor.matmul(
                        at_ps[:], wo_tiles[co][:, cc, :],
                        yTp[:, cc, :],
                        start=(cc == 0), stop=(cc == 7))
                nc.vector.tensor_add(x2w[:, co, :], at_ps[:], x_t[:, co, :])
                nc.sync.dma_start(x2_out[:, co, :], x2w[:, co, :])
                sqf = tl.tile([128, S], f32r, tag="sqf", name=f"sqf{co}",
                              bufs=8)
                nc.scalar.activation(sqf[:], x2w[:, co, :], AF.Square)
                sqfs.append(sqf)
            for co in range(8):
                nc.tensor.matmul(ssq_f[:], ones_col[:], sqfs[co][:],
                                 start=(co == 0), stop=(co == 7))
                nc.tensor.matmul(rt_ps[:], rw_t[:, co, :], x2w[:, co, :],
                                 start=(co == 0), stop=(co == 7))

            srow = tl.tile([1, S], f32, tag="srow", name="srow", bufs=1)
            rrow = tl.tile([1, S], f32r, tag="rrow", name="rrow", bufs=1)
            bcf_sb = tl.tile([128, S], f32, tag="bcfs", name="bcf_sb", bufs=1)
            xfb = x2p.tile([128, 8, S], f32, name="xfb")
            lg = tl.tile([E_MLP + E_VE, S], f32, tag="lg", name="lg", bufs=1)
            for hf in range(2):
                fsl = bass.ts(hf, SW)
                nc.scalar.activation(srow[0:1, fsl], ssq_f[0:1, fsl],
                                     AF.Sqrt, bias=EPS, scale=1.0 / C)
                with nc.allow_low_precision(reason="f32r rms bcast rows"):
                    nc.vector.reciprocal(rrow[0:1, fsl], srow[0:1, fsl])
                bcf_ps = ps_row.tile([128, SW], f32, tag="bcf", name="bcf_ps")
                nc.tensor.matmul(bcf_ps[:], ones_row[:], rrow[0:1, fsl],
                                 start=True, stop=True)
                nc.vector.tensor_copy(bcf_sb[:, fsl], bcf_ps[:])
                nc.vector.tensor_mul(lg[:, fsl], rt_ps[:, fsl],
                                     bcf_sb[0:E_MLP + E_VE, fsl])
                for co in range(8):
                    eng = nc.vector if co % 2 == 0 else nc.gpsimd
                    eng.tensor_mul(xfb[:, co, fsl], x2w[:, co, fsl],
                                   bcf_sb[:, fsl])
                    nc.sync.dma_start(xfb_out[:, co, fsl], xfb[:, co, fsl])
            nc.sync.dma_start(lg_out[:], lg[:])

    nc.compile()
    return nc


# --------------------------------------------------------------------------
# L2: expert MLP (bf16 up, fp8-DR down, gate pre-folded) + VE sum
# --------------------------------------------------------------------------
def build_2(ncap: int):
    nc = bacc.Bacc("TRN2", target_bir_lowering=False, debug=False,
                   num_devices=NCORES)
    NT = ncap // 512

    xfg = nc.dram_tensor("xfg", [128, 8, ncap], bf16,
                         kind="ExternalInput").ap()
    wup = nc.dram_tensor("wup", [128, 8, HID], bf16,
                         kind="ExternalInput").ap()
    wdn = nc.dram_tensor("wdn", [128, 16, C], bf16,
                         kind="ExternalInput").ap()
    ve0 = nc.dram_tensor("ve0", [128, QT, C], fp8e4, kind="ExternalInput").ap()
    ve1 = nc.dram_tensor("ve1", [128, QT, C], fp8e4, kind="ExternalInput").ap()

    moe_out = nc.dram_tensor("moe_out", [128, 8, ncap], bf16,
                             kind="ExternalOutput").ap()
    ve_out = nc.dram_tensor("ve_out", [128, QT, C], bf16,
                            kind="ExternalOutput").ap()

    with tile.TileContext(nc) as tc, ExitStack() as est:
        wp = est.enter_context(tc.tile_pool(name="wp", bufs=1))
        hp = est.enter_context(tc.tile_pool(name="hp", bufs=2))
        op = est.enter_context(tc.tile_pool(name="op", bufs=3))
        ps_h = est.enter_context(tc.tile_pool(name="ps_h", bufs=3, space="PSUM"))
        ps_o = est.enter_context(tc.tile_pool(name="ps_o", bufs=3, space="PSUM"))

        xf_t = wp.tile([128, 8, ncap], bf16, name="xf_t")
        up_t = wp.tile([128, 8, HID], bf16, name="up_t")
        dn_t = wp.tile([128, 16, C], bf16, name="dn_t")
        v0_t = wp.tile([128, QT, C], fp8e4, name="v0_t")
        v1_t = wp.tile([128, QT, C], fp8e4, name="v1_t")
        for cc in range(8):
            nc.sync.dma_start(xf_t[:, cc, :], xfg[:, cc, :])
            nc.sync.dma_start(up_t[:, cc, :], wup[:, cc, :])
        for cc in range(8):
            nc.sync.dma_start(dn_t[:, 2 * cc:2 * cc + 2, :],
                              wdn[:, 2 * cc:2 * cc + 2, :])
        nc.sync.dma_start(v0_t[:], ve0[:])
        nc.sync.dma_start(v1_t[:], ve1[:])

        for nt in range(NT):
            csl = bass.ts(nt, 512)
            h_sb = hp.tile([128, 16, 512], bf16, tag="h", name=f"h{nt}")
            for hc in range(16):
                h_ps = ps_h.tile([128, 512], f32, tag="hps", name="h_ps")
                for cc in range(8):
                    nc.tensor.matmul(
                        h_ps[:], up_t[:, cc, bass.ts(hc, 128)],
                        xf_t[:, cc, csl],
                        start=(cc == 0), stop=(cc == 7))
                hr = op.tile([128, 512], bf16, tag="hr", name="hr")
                nc.scalar.activation(hr[:], h_ps[:], AF.Relu)
                eng = nc.vector if hc % 3 != 2 else nc.gpsimd
                eng.tensor_mul(h_sb[:, hc, :], hr[:], hr[:])
            for co in range(8):
                o_ps = ps_o.tile([128, 512], f32, tag="ops", name="o_ps")
                for hh in range(16):
                    nc.tensor.matmul(
                        o_ps[:], dn_t[:, hh, bass.ts(co, 128)],
                        h_sb[:, hh, :],
                        start=(hh == 0), stop=(hh == 15))
                ot = op.tile([128, 512], bf16, tag="ot", name="ot")
                nc.scalar.copy(ot[:], o_ps[:])
                nc.sync.dma_start(moe_out[:, co, csl], ot[:])

        vo = op.tile([128, QT, C], bf16, tag="vo", name="vo", bufs=1)
        for t in range(QT):
            eng = nc.vector if t % 2 == 0 else nc.gpsimd
            eng.tensor_add(vo[:, t, :], v0_t[:, t, :], v1_t[:, t, :])
        nc.sync.dma_start(ve_out[:], vo[:])

    nc.compile()
    return nc


# --------------------------------------------------------------------------
# Host orchestration
# --------------------------------------------------------------------------
def _qtiles(ci):
    # strip A = 256-tile ci (128-tiles 2ci, 2ci+1), strip B = 256-tile 7-ci
    sa, sb = ci, 7 - ci
    return [2 * sa, 2 * sa + 1, 2 * sb, 2 * sb + 1]


def _slot_map(st, n_s, window):
    """slot -> (key 128-tile, bias).  Diagonal tile (2st+1) at slot n_s-1,
    sub-diagonal (2st) at n_s-2, other causally-alive tiles from slot 0."""
    alive = [kt for kt in range(2 * st)
             if window >= T or 128 * (2 * st + 1 - kt) - 127 <= window]
    m = {}
    for si, kt in enumerate(alive):
        m[si] = (kt, 0.0)
    m[n_s - 2] = (2 * st, 0.0)
    m[n_s - 1] = (2 * st + 1, 0.0)
    return m


def _route(logits, router_bias):
    sig = (1.0 / (1.0 + np.exp(-logits.astype(np.float32)))).astype(np.float32)
    sel = sig + router_bias[None, :].astype(np.float32)
    idx = np.argsort(-sel, axis=1, kind="stable")[:, :TOPK]
    tw = np.take_along_axis(sig, idx, axis=1)
    tw = tw / tw.sum(axis=1, keepdims=True)
    N = logits.shape[0]
    sparse_w = np.zeros((N, E_MLP + E_VE), np.float32)
    np.put_along_axis(sparse_w, idx, tw, axis=1)
    return sparse_w


def kernel(**inputs):
    x = np.asarray(inputs["x"], np.float32)
    token_ids = np.asarray(inputs["token_ids"])
    cos = np.asarray(inputs["cos"], np.float32)
    sin = np.asarray(inputs["sin"], np.float32)
    window = int(np.asarray(inputs["window_size"]))
    wq, wk, wv, wo = (np.asarray(inputs[k], np.float32)
                      for k in ("wq", "wk", "wv", "wo"))
    w_up = np.asarray(inputs["w_up"], np.float32)
    w_down = np.asarray(inputs["w_down"], np.float32)
    router_w = np.asarray(inputs["router_w"], np.float32)
    router_bias = np.asarray(inputs["router_bias"], np.float32)
    ve_tables = np.asarray(inputs["ve_tables"], np.float32)

    cosf = cos[0, :, 0, :]      # (T, 32)
    sinf = sin[0, :, 0, :]

    # ---------------- L1a ----------------
    if "1a" not in _prog_cache:
        _prog_cache["1a"] = build_1a()
    nc1a = _prog_cache["1a"]

    def mov_w(w):  # (M, C) -> [128, 8, M] with [p, cc, m] = w[m, 128cc+p]
        M = w.shape[0]
        return np.ascontiguousarray(
            w.T.reshape(8, 128, M).transpose(1, 0, 2)).astype(np.float32)

    wq_t, wk_t, wv_t = mov_w(wq), mov_w(wk), mov_w(wv)
    toks_all, maps1a = [], []
    for c in range(NCORES):
        b, ci = c // 4, c % 4
        qts = _qtiles(ci)
        toks = np.concatenate([np.arange(qt * 128, qt * 128 + 128)
                               for qt in qts])
        toks_all.append(toks)
        xs = x[b, toks, :]                      # (512, 1024)
        x_fm = np.ascontiguousarray(
            xs.T.reshape(8, 128, S).transpose(1, 0, 2)).astype(np.float32)
        x_tm = np.ascontiguousarray(
            xs.reshape(QT, 128, C).transpose(1, 0, 2)).astype(np.float32)
        cos_tm = np.ascontiguousarray(
            cosf[toks].reshape(QT, 128, 32).transpose(1, 0, 2)
        ).astype(np.float32)
        sin_tm = np.ascontiguousarray(
            sinf[toks].reshape(QT, 128, 32).transpose(1, 0, 2)
        ).astype(np.float32)
        maps1a.append(dict(x_fm=x_fm, x_tm=x_tm, wq_t=wq_t, wk_t=wk_t,
                           wv_t=wv_t, cos_tm=cos_tm, sin_tm=sin_tm))
    res1a = run_bass_kernel_spmd(nc1a, maps1a, list(range(NCORES))).results

    kn = np.zeros((B, T, NKV * HD), np.float32)
    vn = np.zeros((B, T, NKV * HD), np.float32)
    qn = []
    for c in range(NCORES):
        b = c // 4
        toks = toks_all[c]
        kc = res1a[c]["k_out"].astype(np.float32)   # [128, QT, 512]
        vc = res1a[c]["v_out"].astype(np.float32)
        kn[b, toks.reshape(QT, 128)] = kc.transpose(1, 0, 2)
        vn[b, toks.reshape(QT, 128)] = vc.transpose(1, 0, 2)
        qn.append(res1a[c]["q_out"].astype(np.float32))  # [128, QT, 1024]

    # ---------------- L1b ----------------
    masked = window < T
    key1b = ("1b", masked)
    if key1b not in _prog_cache:
        _prog_cache[key1b] = build_1b(masked)
    nc1b = _prog_cache[key1b]

    wo_sc = np.ascontiguousarray(
        wo.T.reshape(8, 128, C).transpose(1, 0, 2)).astype(np.float32)
    rw_sb = np.ascontiguousarray(
        router_w.T.reshape(8, 128, E_MLP + E_VE).transpose(1, 0, 2)
    ).astype(np.float32)

    maps1b = []
    for c in range(NCORES):
        b, ci = c // 4, c % 4
        strips = [ci, 7 - ci]          # 256-token strip indices
        toks = toks_all[c]
        q_sc = np.zeros((65, NH, NSTRIP, SW), np.float32)
        qc = qn[c]                     # [128, QT, 1024] token-major
        qtok = qc.transpose(1, 0, 2).reshape(S, NH, HD)   # (512, NH, 64)
        q_sc[0:64] = qtok.reshape(NSTRIP, SW, NH, HD).transpose(3, 2, 0, 1)
        q_sc[64] = 1.0
        kt_sc = np.zeros((65, NKV, NSLOT_TOT, 128), np.float32)
        kt_sc[64] = DEAD_BIAS
        v_sc = np.zeros((128, NKV, NSLOT_TOT, 65), np.float32)
        v_sc[:, :, :, 64] = 1.0
        kb = kn[b].reshape(16, 128, NKV, HD)   # [ktile, n, kv, d]
        vb = vn[b].reshape(16, 128, NKV, HD)
        for strip in range(NSTRIP):
            st = strips[strip]
            n_s = NPOS[strip]
            smap = _slot_map(st, n_s, window)
            for s, (kt, bias) in smap.items():
                po = POS_BASE[strip] + s
                kt_sc[0:64, :, po, :] = kb[kt].transpose(2, 1, 0)
                kt_sc[64, :, po, :] = bias
                v_sc[:, :, po, 0:64] = vb[kt]
        xs = x[b, toks, :]
        x_fm32 = np.ascontiguousarray(
            xs.T.reshape(8, 128, S).transpose(1, 0, 2)).astype(np.float32)
        m = dict(q_sc=q_sc, kt_sc=np.ascontiguousarray(kt_sc),
                 v_sc=np.ascontiguousarray(v_sc), x_fm32=x_fm32,
                 wo_sc=wo_sc, rw_sb=rw_sb)
        if masked:
            wm = np.zeros((128, NSLOT_TOT * SW), np.float32)
            for strip in range(NSTRIP):
                st = strips[strip]
                n_s = NPOS[strip]
                smap = _slot_map(st, n_s, window)
                for s, (kt, bias) in smap.items():
                    po = POS_BASE[strip] + s
                    qg = 2 * st * 128 + np.arange(SW)[None, :]
                    kg = kt * 128 + np.arange(128)[:, None]
                    ok = (kg <= qg) & (qg - kg <= window)
                    wm[:, po * SW:(po + 1) * SW] = ok
            m["wmask"] = wm.astype(np.float32)
        maps1b.append(m)
    res1b = run_bass_kernel_spmd(nc1b, maps1b, list(range(NCORES))).results

    # ---------------- routing ----------------
    N = B * T
    logits = np.zeros((N, E_MLP + E_VE), np.float32)
    x2 = np.zeros((N, C), np.float32)
    xfb = np.zeros((N, C), np.float32)
    for c in range(NCORES):
        b = c // 4
        toks = toks_all[c] + b * T
        logits[toks] = res1b[c]["lg_out"].T
        x2c = res1b[c]["x2_out"]                     # [128 p, 8 cc, 512 n]
        x2[toks] = x2c.transpose(2, 1, 0).reshape(S, C)
        xfc = res1b[c]["xfb_out"].astype(np.float32)
        xfb[toks] = xfc.transpose(2, 1, 0).reshape(S, C)

    sparse_w = _route(logits, router_bias)

    idx_list = [np.nonzero(sparse_w[:, e])[0] for e in range(E_MLP)]
    n_list = [len(ix) for ix in idx_list]
    ncap = NCAP0
    while ncap < max(n_list):
        ncap += 512

    key2 = ("2", ncap)
    if key2 not in _prog_cache:
        _prog_cache[key2] = build_2(ncap)
    nc2 = _prog_cache[key2]

    tok_flat = token_ids.reshape(-1)
    maps2 = []
    for c in range(NCORES):
        e = c
        ix = idx_list[e]
        g = np.sqrt(sparse_w[ix, e]).astype(np.float32)
        xg = np.zeros((C, ncap), np.float32)
        xg[:, :len(ix)] = xfb[ix].T * g[None, :]
        xfg = np.ascontiguousarray(
            xg.reshape(8, 128, ncap).transpose(1, 0, 2)).astype(BF)
        wup_m = np.ascontiguousarray(
            w_up[e].T.reshape(8, 128, HID).transpose(1, 0, 2)).astype(BF)
        wdn_dr = np.ascontiguousarray(
            w_down[e].T.reshape(16, 128, C).transpose(1, 0, 2)).astype(BF)
        b = c // 4
        toks = toks_all[c] + b * T
        tids = tok_flat[toks]
        ve0 = (64.0 * sparse_w[toks, E_MLP, None]
               * ve_tables[0][tids]).reshape(QT, 128, C).transpose(1, 0, 2)
        ve1 = (64.0 * sparse_w[toks, E_MLP + 1, None]
               * ve_tables[1][tids]).reshape(QT, 128, C).transpose(1, 0, 2)
        maps2.append(dict(xfg=xfg, wup=wup_m, wdn=wdn_dr,
                          ve0=np.ascontiguousarray(ve0).astype(E4),
                          ve1=np.ascontiguousarray(ve1).astype(E4)))
    res2 = run_bass_kernel_spmd(nc2, maps2, list(range(NCORES))).results

    out = x2.copy()
    for c in range(NCORES):
        b = c // 4
        toks = toks_all[c] + b * T
        veo = res2[c]["ve_out"].astype(np.float32)  # [128, QT, C]
        out[toks] += veo.transpose(1, 0, 2).reshape(S, C) / 64.0
    for e in range(E_MLP):
        n_e = n_list[e]
        if n_e:
            moe = res2[e]["moe_out"].astype(np.float32)  # [128, 8, ncap]
            out[idx_list[e]] += moe[:, :, :n_e].transpose(
                2, 1, 0).reshape(n_e, C)
    return out.reshape(B, T, C).astype(np.float32)


# revision 48
# speedup vs baseline: 1.0180x; 1.0077x over previous
"""TRN2 Bass kernel for nn_BlockMoVaE (attention + MoE/VE routing block).

Self-contained: accepts FULL inputs, shards across 8 NeuronCores, returns
FULL output.

Three SPMD launches with host re-layout between them (host does only data
movement / routing; all FLOPs stay on device).  The router top-2 decision
is discrete, so everything feeding the logits (all of phase 1) runs at
f32r precision; only the post-routing expert MLP uses bf16/fp8.

  L1a  (token-parallel, 512 tokens/core = 2 causally-balanced 256-strips):
       x-rms stats, raw-x f32r QKV projections, token-major rope +
       per-head rmsnorm (the per-token x-norm scalar cancels inside the
       head rmsnorm, so Q/K project raw x; V is scaled by r explicitly).
       Exports token-major q/k/v (f32).
  L1b  (token-parallel attention): host re-lays q/k/v into score-friendly
       f32r layouts.  Causal work is balanced by giving core ci the
       256-token strips {ci, 7-ci} of its batch; the static program
       computes (8, 16) key-128-slots for the two strips; fully-dead
       slots are killed by a rank-1 bias row (-30000) folded into the
       score matmul contraction, diagonal tiles by static affine_selects.
       Softmax denominators ride along as a ones-column of V.  Then
       wo + residual + xf rmsnorm + router logits.
  L2   (expert-parallel MoE): host routes top-2 and gathers tokens per
       expert with sqrt(gate) pre-scaling (relu^2 is 2-homogeneous so the
       gate factors exactly); bf16 up / fp8-DoubleRow down projections;
       VE rows host-gathered, gate-pre-scaled, summed on device.
"""
import numpy as np
import ml_dtypes

import concourse.bass as bass
import concourse.bacc as bacc
import concourse.mybir as mybir
import concourse.tile as tile
from concourse.bass_utils import run_bass_kernel_spmd
from concourse.alu_op_type import AluOpType
from contextlib import ExitStack
from collections import deque

# ---- problem constants (hardcoded per contest rules) ----
B, T, C = 2, 2048, 1024
NH, NKV, HD = 16, 8, 64
E_MLP, E_VE, TOPK = 8, 2, 2
HID = 2048
VOCAB = 50257
EPS = 1e-6
NCORES = 8
S = 512              # tokens per core
QT = 4               # 128-token tiles per core (2 strips of 256)
NSTRIP = 2
SW = 256             # strip width (queries)
NPOS = (8, 16)       # static key-slot count per strip
POS_BASE = (0, 8)    # slot base in kT layout (total 24)
NSLOT_TOT = 24
DEAD_BIAS = -30000.0
NCAP0 = 1024

f32 = mybir.dt.float32
f32r = mybir.dt.float32r
bf16 = mybir.dt.bfloat16
fp8e4 = mybir.dt.float8e4
AF = mybir.ActivationFunctionType
DR = mybir.MatmulPerfMode.DoubleRow
E4 = ml_dtypes.float8_e4m3
BF = ml_dtypes.bfloat16

_prog_cache = {}


def _register_consts(nc, values):
    for value in values:
        key = (f32, float(value))
        if key not in nc.const_aps.aps:
            t = nc.alloc_sbuf_tensor(f"constap-{value}", [128, 1], f32)
            nc.gpsimd.memset(t.ap(), float(value))
            nc.const_aps.aps[key] = t.ap()
    nc.all_engine_barrier()


# --------------------------------------------------------------------------
# L1a: x stats + QKV projection + rope + head-rms (token-major epilogues)
# --------------------------------------------------------------------------
def build_1a():
    nc = bacc.Bacc("TRN2", target_bir_lowering=False, debug=False,
                   num_devices=NCORES)

    x_fm = nc.dram_tensor("x_fm", [128, 8, S], f32r, kind="ExternalInput").ap()
    x_tm = nc.dram_tensor("x_tm", [128, QT, C], f32, kind="ExternalInput").ap()
    wq_t = nc.dram_tensor("wq_t", [128, 8, NH * HD], f32r,
                          kind="ExternalInput").ap()
    wk_t = nc.dram_tensor("wk_t", [128, 8, NKV * HD], f32r,
                          kind="ExternalInput").ap()
    wv_t = nc.dram_tensor("wv_t", [128, 8, NKV * HD], f32r,
                          kind="ExternalInput").ap()
    cos_tm = nc.dram_tensor("cos_tm", [128, QT, 32], f32,
                            kind="ExternalInput").ap()
    sin_tm = nc.dram_tensor("sin_tm", [128, QT, 32], f32,
                            kind="ExternalInput").ap()

    q_out = nc.dram_tensor("q_out", [128, QT, NH * HD], f32,
                           kind="ExternalOutput").ap()
    k_out = nc.dram_tensor("k_out", [128, QT, NKV * HD], f32,
                           kind="ExternalOutput").ap()
    v_out = nc.dram_tensor("v_out", [128, QT, NKV * HD], f32,
                           kind="ExternalOutput").ap()

    _register_consts(nc, [EPS])
    with tile.TileContext(nc) as tc, ExitStack() as est:
        wp = est.enter_context(tc.tile_pool(name="wp", bufs=1))
        work = est.enter_context(tc.tile_pool(name="work", bufs=2))
        outp = est.enter_context(tc.tile_pool(name="outp", bufs=1))
        ps_q = est.enter_context(tc.tile_pool(name="ps_q", bufs=2, space="PSUM"))
        ps_k = est.enter_context(tc.tile_pool(name="ps_k", bufs=2, space="PSUM"))
        ps_v = est.enter_context(tc.tile_pool(name="ps_v", bufs=2, space="PSUM"))

        xf = wp.tile([128, 8, S], f32r, name="xf")
        xt = wp.tile([128, QT, C], f32, name="xt")
        wq = wp.tile([128, 8, NH * HD], f32r, name="wq")
        wk = wp.tile([128, 8, NKV * HD], f32r, name="wk")
        wv = wp.tile([128, 8, NKV * HD], f32r, name="wv")
        cs = wp.tile([128, QT, 32], f32, name="cs")
        sn = wp.tile([128, QT, 32], f32, name="sn")
        for cc in range(8):
            nc.sync.dma_start(xf[:, cc, :], x_fm[:, cc, :])
            nc.sync.dma_start(wq[:, cc, :], wq_t[:, cc, :])
        for cc in range(8):
            nc.sync.dma_start(wk[:, cc, :], wk_t[:, cc, :])
            nc.sync.dma_start(wv[:, cc, :], wv_t[:, cc, :])
        nc.sync.dma_start(cs[:], cos_tm[:])
        nc.sync.dma_start(sn[:], sin_tm[:])
        nc.sync.dma_start(xt[:], x_tm[:])

        qe = outp.tile([128, QT, NH * HD], f32, name="qe")
        ke = outp.tile([128, QT, NKV * HD], f32, name="ke")
        ve = outp.tile([128, QT, NKV * HD], f32, name="ve")

        def rope_norm(ps, nh, t, out_tile):
            """Token-major rope + per-head rmsnorm from psum [128, nh*64]."""
            qs = work.tile([128, nh, HD], f32, tag=f"qs{nh}", name="qs")
            nc.scalar.copy(qs[:], ps[:].rearrange("p (h d) -> p h d", d=HD))
            cosb = cs[:, t:t + 1, :].broadcast_to([128, nh, 32])
            sinb = sn[:, t:t + 1, :].broadcast_to([128, nh, 32])
            rp = work.tile([128, nh, HD], f32, tag=f"rp{nh}", name="rp")
            a = work.tile([128, nh, 32], f32, tag=f"ra{nh}", name="ra")
            b = work.tile([128, nh, 32], f32, tag=f"rb{nh}", name="rb")
            c2 = work.tile([128, nh, 32], f32, tag=f"rc{nh}", name="rc")
            d2 = work.tile([128, nh, 32], f32, tag=f"rd{nh}", name="rd")
            nc.vector.tensor_mul(a[:], qs[:, :, 0:32], cosb)
            nc.vector.tensor_mul(b[:], qs[:, :, 32:64], sinb)
            nc.gpsimd.tensor_mul(c2[:], qs[:, :, 32:64], cosb)
            nc.gpsimd.tensor_mul(d2[:], qs[:, :, 0:32], sinb)
            nc.gpsimd.tensor_add(rp[:, :, 0:32], a[:], b[:])
            nc.vector.tensor_sub(rp[:, :, 32:64], c2[:], d2[:])
            sq = work.tile([128, nh, HD], f32, tag=f"sq{nh}", name="sq")
            nc.scalar.activation(sq[:], rp[:], AF.Square)
            hs = work.tile([128, nh, 1], f32, tag=f"hs{nh}", name="hs")
            nc.vector.tensor_reduce(out=hs[:], in_=sq[:], op=AluOpType.add,
                                    axis=mybir.AxisListType.X)
            sh = work.tile([128, nh, 1], f32, tag=f"sh{nh}", name="sh")
            nc.scalar.activation(sh[:], hs[:], AF.Sqrt, bias=EPS,
                                 scale=1.0 / HD)
            rh = work.tile([128, nh, 1], f32, tag=f"rh{nh}", name="rh")
            with nc.allow_low_precision(reason="head rms recip"):
                nc.vector.reciprocal(rh[:], sh[:])
            nc.vector.tensor_mul(
                out_tile[:].rearrange("p (h d) -> p h d", d=HD),
                rp[:], rh[:].broadcast_to([128, nh, HD]))

        for t in range(QT):
            # per-token inv-rms of x (V only; cancels inside Q/K head-rms)
            xsq = work.tile([128, C], f32, tag="xsq", name="xsq")
            nc.scalar.activation(xsq[:], xt[:, t, :], AF.Square)
            ssq = work.tile([128, 1], f32, tag="ssq", name="ssq")
            nc.vector.tensor_reduce(out=ssq[:], in_=xsq[:], op=AluOpType.add,
                                    axis=mybir.AxisListType.XYZW)
            sx = work.tile([128, 1], f32, tag="sx", name="sx")
            nc.scalar.activation(sx[:], ssq[:], AF.Sqrt, bias=EPS,
                                 scale=1.0 / C)
            r = work.tile([128, 1], f32, tag="r", name="r")
            with nc.allow_low_precision(reason="x rms recip"):
                nc.vector.reciprocal(r[:], sx[:])

            q_ps = ps_q.tile([128, NH * HD], f32, name="q_ps")
            k_ps = ps_k.tile([128, NKV * HD], f32, name="k_ps")
            v_ps = ps_v.tile([128, NKV * HD], f32, name="v_ps")
            for half in range(2):
                hsl = bass.ts(half, NH * HD // 2)
                for cc in range(8):
                    nc.tensor.matmul(q_ps[:, hsl],
                                     xf[:, cc, bass.ts(t, 128)],
                                     wq[:, cc, hsl],
                                     start=(cc == 0), stop=(cc == 7))
            for cc in range(8):
                nc.tensor.matmul(k_ps[:], xf[:, cc, bass.ts(t, 128)],
                                 wk[:, cc, :], start=(cc == 0), stop=(cc == 7))
            for cc in range(8):
                nc.tensor.matmul(v_ps[:], xf[:, cc, bass.ts(t, 128)],
                                 wv[:, cc, :], start=(cc == 0), stop=(cc == 7))

            rope_norm(q_ps, NH, t, qe[:, t, :])
            nc.sync.dma_start(q_out[:, t, :], qe[:, t, :])
            rope_norm(k_ps, NKV, t, ke[:, t, :])
            nc.sync.dma_start(k_out[:, t, :], ke[:, t, :])
            nc.vector.tensor_scalar_mul(ve[:, t, :], v_ps[:], r[:])
            nc.sync.dma_start(v_out[:, t, :], ve[:, t, :])

    nc.compile()
    return nc


# --------------------------------------------------------------------------
# L1b: attention + wo + residual + xf rmsnorm + router logits
# --------------------------------------------------------------------------
def build_1b(masked: bool):
    nc = bacc.Bacc("TRN2", target_bir_lowering=False, debug=False,
                   num_devices=NCORES)

    # q rows 0..63 = head dims, row 64 = 1.0 (rank-1 bias carrier)
    q_sc = nc.dram_tensor("q_sc", [65, NH, NSTRIP, SW], f32r,
                          kind="ExternalInput").ap()
    kt_sc = nc.dram_tensor("kt_sc", [65, NKV, NSLOT_TOT, 128], f32r,
                           kind="ExternalInput").ap()
    # v columns 0..63 = v dims, col 64 = 1.0 (softmax denominator)
    v_sc = nc.dram_tensor("v_sc", [128, NKV, NSLOT_TOT, 65], f32r,
                          kind="ExternalInput").ap()
    x_fm32 = nc.dram_tensor("x_fm32", [128, 8, S], f32,
                            kind="ExternalInput").ap()
    wo_sc = nc.dram_tensor("wo_sc", [128, 8, C], f32r,
                           kind="ExternalInput").ap()
    rw_sb = nc.dram_tensor("rw_sb", [128, 8, E_MLP + E_VE], f32,
                           kind="ExternalInput").ap()
    if masked:
        wmask = nc.dram_tensor("wmask", [128, NSLOT_TOT * SW], f32,
                               kind="ExternalInput").ap()

    x2_out = nc.dram_tensor("x2_out", [128, 8, S], f32,
                            kind="ExternalOutput").ap()
    xfb_out = nc.dram_tensor("xfb_out", [128, 8, S], bf16,
                             kind="ExternalOutput").ap()
    lg_out = nc.dram_tensor("lg_out", [E_MLP + E_VE, S], f32,
                            kind="ExternalOutput").ap()

    _register_consts(nc, [EPS])
    with tile.TileContext(nc) as tc, ExitStack() as est:
        wp = est.enter_context(tc.tile_pool(name="wp", bufs=1))
        ytp = est.enter_context(tc.tile_pool(name="ytp", bufs=1))

        yTp = ytp.tile([128, NH // 2, S], f32r, name="yTp")
        yTo = ytp.tile([64, NH // 2, S], f32r, name="yTo")

        with tc.tile_pool(name="ps_sc", bufs=2, space="PSUM") as ps_sc, \
             tc.tile_pool(name="ps_yv", bufs=1, space="PSUM") as ps_yv, \
             tc.tile_pool(name="ps_bc", bufs=1, space="PSUM") as ps_bc, \
             tc.tile_pool(name="attp", bufs=1) as attp, \
             tc.tile_pool(name="kvs", bufs=2) as kvs, \
             tc.tile_pool(name="ptp", bufs=9) as ptp, \
             tc.tile_pool(name="ivp", bufs=2) as ivp:
            q_t = attp.tile([65, NH, NSTRIP, SW], f32r, name="q_t")
            ones64f = attp.tile([1, 64], f32, name="ones64f")
            nc.vector.memset(ones64f[:], 1.0)
            ones64 = attp.tile([1, 64], f32r, name="ones64")
            nc.scalar.copy(ones64[:], ones64f[:])
            if masked:
                wm_t = attp.tile([128, NSLOT_TOT * SW], f32, name="wm_t")
                nc.sync.dma_start(wm_t[:], wmask[:])

            # stream q/kt/v per (strip, kv-pair); each slice loaded once
            kv_tiles = {}
            for strip in range(NSTRIP):
                n_s = NPOS[strip]
                for kp in range(4):
                    nc.sync.dma_start(
                        q_t[:, 4 * kp:4 * kp + 4, strip, :],
                        q_sc[:, 4 * kp:4 * kp + 4, strip, :])
                    kt = kvs.tile([65, 2, n_s, 128], f32r,
                                  tag=f"kt{strip}", name=f"kt{strip}_{kp}")
                    sl = slice(POS_BASE[strip], POS_BASE[strip] + n_s)
                    nc.sync.dma_start(kt[:], kt_sc[:, 2 * kp:2 * kp + 2, sl, :])
                    vt = kvs.tile([128, 2, n_s, 65], f32r,
                                  tag=f"vt{strip}", name=f"vt{strip}_{kp}")
                    nc.sync.dma_start(vt[:], v_sc[:, 2 * kp:2 * kp + 2, sl, :])
                    kv_tiles[(strip, kp)] = (kt, vt)
            x_t = wp.tile([128, 8, S], f32, name="x_t")
            rw_t = wp.tile([128, 8, E_MLP + E_VE], f32, name="rw_t")
            nc.sync.dma_start(x_t[:], x_fm32[:])
            nc.sync.dma_start(rw_t[:], rw_sb[:])

            def emit_scores(strip, hg):
                """Scores + exp + mask for the 4 heads of kv-pair hg."""
                n_s = NPOS[strip]
                kt, _ = kv_tiles[(strip, hg)]
                chunks = [(c0, min(4, n_s - c0)) for c0 in range(0, n_s, 4)]
                pts = []
                for hi in range(4):
                    h = 4 * hg + hi
                    kvl = hi // 2          # kv head within the pair
                    pt_chunks = []
                    for c0, cn in chunks:
                        sc = ps_sc.tile([128, 4 * SW], f32, tag="sc",
                                        name="sc")
                        for s in range(cn):
                            nc.tensor.matmul(
                                sc[:, bass.ts(s, SW)],
                                kt[:, kvl, c0 + s, :],
                                q_t[:, h, strip, :],
                                start=True, stop=True)
                        pt = ptp.tile([128, 4 * SW], f32r, tag="pt",
                                      name=f"pt{strip}_{h}_{c0}")
                        nc.scalar.activation(pt[:, 0:cn * SW],
                                             sc[:, 0:cn * SW],
                                             AF.Exp, scale=0.125)
                        if masked:
                            base = (POS_BASE[strip] + c0) * SW
                            nc.vector.tensor_mul(
                                pt[:, 0:cn * SW], pt[:, 0:cn * SW],
                                wm_t[:, base:base + cn * SW])
                        else:
                            if c0 + cn == n_s:
                                # main diagonal tile (last slot): q-half 0
                                # is fully future -> zero; q-half 1: k <= q
                                off = (cn - 1) * SW
                                zsl = pt[:, off:off + 128]
                                nc.gpsimd.affine_select(
                                    zsl, zsl, pattern=[[1, 128]], base=-128,
                                    channel_multiplier=-1,
                                    compare_op=AluOpType.is_ge, fill=0.0)
                                dsl = pt[:, off + 128:off + 256]
                                nc.gpsimd.affine_select(
                                    dsl, dsl, pattern=[[1, 128]], base=0,
                                    channel_multiplier=-1,
                                    compare_op=AluOpType.is_ge, fill=0.0)
                                # sub-diagonal (slot n_s-2): q-half 0 k<=q
                                if cn >= 2:
                                    ssl = pt[:, off - SW:off - 128]
                                    nc.gpsimd.affine_select(
                                        ssl, ssl, pattern=[[1, 128]], base=0,
                                        channel_multiplier=-1,
                                        compare_op=AluOpType.is_ge, fill=0.0)
                        pt_chunks.append(pt)
                    pts.append(pt_chunks)
                return pts, chunks

            def emit_yv(strip, hg, pts, chunks):
                """p@v accumulate (ones col 64 -> den row 64) + normalize."""
                n_s = NPOS[strip]
                _, vt = kv_tiles[(strip, hg)]
                yv_ps = ps_yv.tile([65, 4 * SW], f32, tag="yv",
                                   name=f"yv{strip}_{hg}")
                for hi in range(4):
                    kvl = hi // 2
                    for (c0, cn), pt in zip(chunks, pts[hi]):
                        for s in range(cn):
                            nc.tensor.matmul(
                                yv_ps[:, bass.ts(hi, SW)],
                                vt[:, kvl, c0 + s, :],
                                pt[:, bass.ts(s, SW)],
                                start=(c0 + s == 0),
                                stop=(c0 + s == n_s - 1))
                iv = ivp.tile([1, 4 * SW], f32r, tag="iv", name="iv")
                with nc.allow_low_precision(reason="softmax recip"):
                    nc.vector.reciprocal(iv[:], yv_ps[64:65, :])
                bc_ps = ps_bc.tile([64, 4 * SW], f32, tag="bc", name="bc")
                for hi in range(4):
                    nc.tensor.matmul(bc_ps[:, bass.ts(hi, SW)],
                                     ones64[:], iv[0:1, bass.ts(hi, SW)],
                                     start=True, stop=True)
                bc_sb = ivp.tile([64, 4 * SW], f32, tag="bcs", name="bcs")
                nc.vector.tensor_copy(bc_sb[:], bc_ps[:])
                yv4 = yv_ps[0:64, :].rearrange("p (h n) -> p h n", h=4)
                bc4 = bc_sb[:].rearrange("p (h n) -> p h n", h=4)
                ssl = bass.ts(strip, SW)
                # even heads (hi 0,2) -> chunks 2hg..2hg+1 rows 0:64
                nc.vector.tensor_mul(
                    yTp[0:64, 2 * hg:2 * hg + 2, ssl],
                    yv4[:, 0:4:2, :], bc4[:, 0:4:2, :])
                # odd heads -> staging, then partition-shift DMA
                nc.vector.tensor_mul(
                    yTo[:, 2 * hg:2 * hg + 2, ssl],
                    yv4[:, 1:4:2, :], bc4[:, 1:4:2, :])
                nc.sync.dma_start(yTp[64:128, 2 * hg:2 * hg + 2, ssl],
                                  yTo[:, 2 * hg:2 * hg + 2, ssl])

            pending = deque()
            LAG = 2
            for strip in range(NSTRIP):
                for hg in range(4):
                    pts, chunks = emit_scores(strip, hg)
                    pending.append((strip, hg, pts, chunks))
                    if len(pending) > LAG:
                        emit_yv(*pending.popleft())
            while pending:
                emit_yv(*pending.popleft())

        # ---- wo + residual + xf rmsnorm + router ----
        with tc.tile_pool(name="ps_at", bufs=2, space="PSUM") as ps_at, \
             tc.tile_pool(name="ps_row", bufs=2, space="PSUM") as ps_row, \
             tc.tile_pool(name="ps_bcf", bufs=1, space="PSUM") as ps_bcf, \
             tc.tile_pool(name="tl", bufs=2) as tl, \
             tc.tile_pool(name="x2p", bufs=1) as x2p:
            ones_f = tl.tile([128, 1], f32, tag="onesf", name="ones_f", bufs=1)
            nc.vector.memset(ones_f[:], 1.0)
            ones_col = tl.tile([128, 1], f32r, tag="onesc", name="ones_col",
                               bufs=1)
            nc.scalar.copy(ones_col[:], ones_f[:])
            ones_rf = tl.tile([1, 128], f32, tag="onesrf", name="ones_rf",
                              bufs=1)
            nc.vector.memset(ones_rf[:], 1.0)
            ones_row = tl.tile([1, 128], f32r, tag="onesr", name="ones_row",
                               bufs=1)
            nc.scalar.copy(ones_row[:], ones_rf[:])

            x2w = x2p.tile([128, 8, S], f32, name="x2w")
            ssq_f = ps_bcf.tile([1, S], f32, tag="ssqf", name="ssq_f")
            rt_ps = ps_row.tile([E_MLP + E_VE, S], f32, tag="rt", name="rt_ps")
            wo_tiles = []
            for co in range(8):
                wo_t = tl.tile([128, 8, 128], f32r, tag="wo",
                               name=f"wo{co}", bufs=5)
                nc.sync.dma_start(wo_t[:], wo_sc[:, :, bass.ts(co, 128)])
                wo_tiles.append(wo_t)
            sqfs = []
            for co in range(8):
                at_ps = ps_at.tile([128, S], f32, tag="at", name="at_ps")
                for cc in range(8):
                    nc.tensor.matmul(
                        at_ps[:], wo_tiles[co][:, cc, :],
                        yTp[:, cc, :],
                        start=(cc == 0), stop=(cc == 7))
                nc.vector.tensor_add(x2w[:, co, :], at_ps[:], x_t[:, co, :])
                nc.sync.dma_start(x2_out[:, co, :], x2w[:, co, :])
                sqf = tl.tile([128, S], f32r, tag="sqf", name=f"sqf{co}",
                              bufs=8)
                nc.scalar.activation(sqf[:], x2w[:, co, :], AF.Square)
                sqfs.append(sqf)
            for co in range(8):
                nc.tensor.matmul(ssq_f[:], ones_col[:], sqfs[co][:],
                                 start=(co == 0), stop=(co == 7))
                nc.tensor.matmul(rt_ps[:], rw_t[:, co, :], x2w[:, co, :],
                                 start=(co == 0), stop=(co == 7))

            srow = tl.tile([1, S], f32, tag="srow", name="srow", bufs=1)
            rrow = tl.tile([1, S], f32r, tag="rrow", name="rrow", bufs=1)
            bcf_sb = tl.tile([128, S], f32, tag="bcfs", name="bcf_sb", bufs=1)
            xfb = x2p.tile([128, 8, S], bf16, name="xfb")
            lg = tl.tile([E_MLP + E_VE, S], f32, tag="lg", name="lg", bufs=1)
            for hf in range(2):
                fsl = bass.ts(hf, SW)
                nc.scalar.activation(srow[0:1, fsl], ssq_f[0:1, fsl],
                                     AF.Sqrt, bias=EPS, scale=1.0 / C)
                with nc.allow_low_precision(reason="f32r rms bcast rows"):
                    nc.vector.reciprocal(rrow[0:1, fsl], srow[0:1, fsl])
                bcf_ps = ps_row.tile([128, SW], f32, tag="bcf", name="bcf_ps")
                nc.tensor.matmul(bcf_ps[:], ones_row[:], rrow[0:1, fsl],
                                 start=True, stop=True)
                nc.vector.tensor_copy(bcf_sb[:, fsl], bcf_ps[:])
                nc.vector.tensor_mul(lg[:, fsl], rt_ps[:, fsl],
                                     bcf_sb[0:E_MLP + E_VE, fsl])
                for co in range(8):
                    eng = nc.vector if co % 2 == 0 else nc.gpsimd
                    eng.tensor_mul(xfb[:, co, fsl], x2w[:, co, fsl],
                                   bcf_sb[:, fsl])
                    nc.sync.dma_start(xfb_out[:, co, fsl], xfb[:, co, fsl])
            nc.sync.dma_start(lg_out[:], lg[:])

    nc.compile()
    return nc


# --------------------------------------------------------------------------
# L2: expert MLP (bf16 up, fp8-DR down, gate pre-folded) + VE sum
# --------------------------------------------------------------------------
def build_2(ncap: int):
    nc = bacc.Bacc("TRN2", target_bir_lowering=False, debug=False,
                   num_devices=NCORES)
    NT = ncap // 512

    xfg = nc.dram_tensor("xfg", [128, 8, ncap], bf16,
                         kind="ExternalInput").ap()
    wup = nc.dram_tensor("wup", [128, 8, HID], bf16,
                         kind="ExternalInput").ap()
    wdn = nc.dram_tensor("wdn", [128, 16, C], bf16,
                         kind="ExternalInput").ap()
    ve0 = nc.dram_tensor("ve0", [128, QT, C], fp8e4, kind="ExternalInput").ap()
    ve1 = nc.dram_tensor("ve1", [128, QT, C], fp8e4, kind="ExternalInput").ap()

    moe_out = nc.dram_tensor("moe_out", [128, 8, ncap], bf16,
                             kind="ExternalOutput").ap()
    ve_out = nc.dram_tensor("ve_out", [128, QT, C], bf16,
                            kind="ExternalOutput").ap()

    with tile.TileContext(nc) as tc, ExitStack() as est:
        wp = est.enter_context(tc.tile_pool(name="wp", bufs=1))
        hp = est.enter_context(tc.tile_pool(name="hp", bufs=2))
        op = est.enter_context(tc.tile_pool(name="op", bufs=3))
        ps_h = est.enter_context(tc.tile_pool(name="ps_h", bufs=3, space="PSUM"))
        ps_o = est.enter_context(tc.tile_pool(name="ps_o", bufs=3, space="PSUM"))

        xf_t = wp.tile([128, 8, ncap], bf16, name="xf_t")
        up_t = wp.tile([128, 8, HID], bf16, name="up_t")
        dn_t = wp.tile([128, 16, C], bf16, name="dn_t")
        v0_t = wp.tile([128, QT, C], fp8e4, name="v0_t")
        v1_t = wp.tile([128, QT, C], fp8e4, name="v1_t")
        for cc in range(8):
            nc.sync.dma_start(xf_t[:, cc, :], xfg[:, cc, :])
            nc.sync.dma_start(up_t[:, cc, :], wup[:, cc, :])
        for cc in range(8):
            nc.sync.dma_start(dn_t[:, 2 * cc:2 * cc + 2, :],
                              wdn[:, 2 * cc:2 * cc + 2, :])
        nc.sync.dma_start(v0_t[:], ve0[:])
        nc.sync.dma_start(v1_t[:], ve1[:])

        for nt in range(NT):
            csl = bass.ts(nt, 512)
            h_sb = hp.tile([128, 16, 512], bf16, tag="h", name=f"h{nt}")
            for hc in range(16):
                h_ps = ps_h.tile([128, 512], f32, tag="hps", name="h_ps")
                for cc in range(8):
                    nc.tensor.matmul(
                        h_ps[:], up_t[:, cc, bass.ts(hc, 128)],
                        xf_t[:, cc, csl],
                        start=(cc == 0), stop=(cc == 7))
                hr = op.tile([128, 512], bf16, tag="hr", name="hr")
                nc.scalar.activation(hr[:], h_ps[:], AF.Relu)
                eng = nc.vector if hc % 3 != 2 else nc.gpsimd
                eng.tensor_mul(h_sb[:, hc, :], hr[:], hr[:])
            for co in range(8):
                o_ps = ps_o.tile([128, 512], f32, tag="ops", name="o_ps")
                for hh in range(16):
                    nc.tensor.matmul(
                        o_ps[:], dn_t[:, hh, bass.ts(co, 128)],
                        h_sb[:, hh, :],
                        start=(hh == 0), stop=(hh == 15))
                ot = op.tile([128, 512], bf16, tag="ot", name="ot")
                nc.scalar.copy(ot[:], o_ps[:])
                nc.sync.dma_start(moe_out[:, co, csl], ot[:])

        vo = op.tile([128, QT, C], bf16, tag="vo", name="vo", bufs=1)
        for t in range(QT):
            eng = nc.vector if t % 2 == 0 else nc.gpsimd
            eng.tensor_add(vo[:, t, :], v0_t[:, t, :], v1_t[:, t, :])
        nc.sync.dma_start(ve_out[:], vo[:])

    nc.compile()
    return nc


# --------------------------------------------------------------------------
# Host orchestration
# --------------------------------------------------------------------------
def _qtiles(ci):
    # strip A = 256-tile ci (128-tiles 2ci, 2ci+1), strip B = 256-tile 7-ci
    sa, sb = ci, 7 - ci
    return [2 * sa, 2 * sa + 1, 2 * sb, 2 * sb + 1]


def _slot_map(st, n_s, window):
    """slot -> (key 128-tile, bias).  Diagonal tile (2st+1) at slot n_s-1,
    sub-diagonal (2st) at n_s-2, other causally-alive tiles from slot 0."""
    alive = [kt for kt in range(2 * st)
             if window >= T or 128 * (2 * st + 1 - kt) - 127 <= window]
    m = {}
    for si, kt in enumerate(alive):
        m[si] = (kt, 0.0)
    m[n_s - 2] = (2 * st, 0.0)
    m[n_s - 1] = (2 * st + 1, 0.0)
    return m


def _route(logits, router_bias):
    sig = (1.0 / (1.0 + np.exp(-logits.astype(np.float32)))).astype(np.float32)
    sel = sig + router_bias[None, :].astype(np.float32)
    idx = np.argsort(-sel, axis=1, kind="stable")[:, :TOPK]
    tw = np.take_along_axis(sig, idx, axis=1)
    tw = tw / tw.sum(axis=1, keepdims=True)
    N = logits.shape[0]
    sparse_w = np.zeros((N, E_MLP + E_VE), np.float32)
    np.put_along_axis(sparse_w, idx, tw, axis=1)
    return sparse_w


def kernel(**inputs):
    x = np.asarray(inputs["x"], np.float32)
    token_ids = np.asarray(inputs["token_ids"])
    cos = np.asarray(inputs["cos"], np.float32)
    sin = np.asarray(inputs["sin"], np.float32)
    window = int(np.asarray(inputs["window_size"]))
    wq, wk, wv, wo = (np.asarray(inputs[k], np.float32)
                      for k in ("wq", "wk", "wv", "wo"))
    w_up = np.asarray(inputs["w_up"], np.float32)
    w_down = np.asarray(inputs["w_down"], np.float32)
    router_w = np.asarray(inputs["router_w"], np.float32)
    router_bias = np.asarray(inputs["router_bias"], np.float32)
    ve_tables = np.asarray(inputs["ve_tables"], np.float32)

    cosf = cos[0, :, 0, :]      # (T, 32)
    sinf = sin[0, :, 0, :]

    # ---------------- L1a ----------------
    if "1a" not in _prog_cache:
        _prog_cache["1a"] = build_1a()
    nc1a = _prog_cache["1a"]

    def mov_w(w):  # (M, C) -> [128, 8, M] with [p, cc, m] = w[m, 128cc+p]
        M = w.shape[0]
        return np.ascontiguousarray(
            w.T.reshape(8, 128, M).transpose(1, 0, 2)).astype(np.float32)

    wq_t, wk_t, wv_t = mov_w(wq), mov_w(wk), mov_w(wv)
    toks_all, maps1a = [], []
    for c in range(NCORES):
        b, ci = c // 4, c % 4
        qts = _qtiles(ci)
        toks = np.concatenate([np.arange(qt * 128, qt * 128 + 128)
                               for qt in qts])
        toks_all.append(toks)
        xs = x[b, toks, :]                      # (512, 1024)
        x_fm = np.ascontiguousarray(
            xs.T.reshape(8, 128, S).transpose(1, 0, 2)).astype(np.float32)
        x_tm = np.ascontiguousarray(
            xs.reshape(QT, 128, C).transpose(1, 0, 2)).astype(np.float32)
        cos_tm = np.ascontiguousarray(
            cosf[toks].reshape(QT, 128, 32).transpose(1, 0, 2)
        ).astype(np.float32)
        sin_tm = np.ascontiguousarray(
            sinf[toks].reshape(QT, 128, 32).transpose(1, 0, 2)
        ).astype(np.float32)
        maps1a.append(dict(x_fm=x_fm, x_tm=x_tm, wq_t=wq_t, wk_t=wk_t,
                           wv_t=wv_t, cos_tm=cos_tm, sin_tm=sin_tm))
    res1a = run_bass_kernel_spmd(nc1a, maps1a, list(range(NCORES))).results

    kn = np.zeros((B, T, NKV * HD), np.float32)
    vn = np.zeros((B, T, NKV * HD), np.float32)
    qn = []
    for c in range(NCORES):
        b = c // 4
        toks = toks_all[c]
        kc = res1a[c]["k_out"].astype(np.float32)   # [128, QT, 512]
        vc = res1a[c]["v_out"].astype(np.float32)
        kn[b, toks.reshape(QT, 128)] = kc.transpose(1, 0, 2)
        vn[b, toks.reshape(QT, 128)] = vc.transpose(1, 0, 2)
        qn.append(res1a[c]["q_out"].astype(np.float32))  # [128, QT, 1024]

    # ---------------- L1b ----------------
    masked = window < T
    key1b = ("1b", masked)
    if key1b not in _prog_cache:
        _prog_cache[key1b] = build_1b(masked)
    nc1b = _prog_cache[key1b]

    wo_sc = np.ascontiguousarray(
        wo.T.reshape(8, 128, C).transpose(1, 0, 2)).astype(np.float32)
    rw_sb = np.ascontiguousarray(
        router_w.T.reshape(8, 128, E_MLP + E_VE).transpose(1, 0, 2)
    ).astype(np.float32)

    maps1b = []
    for c in range(NCORES):
        b, ci = c // 4, c % 4
        strips = [ci, 7 - ci]          # 256-token strip indices
        toks = toks_all[c]
        q_sc = np.zeros((65, NH, NSTRIP, SW), np.float32)
        qc = qn[c]                     # [128, QT, 1024] token-major
        qtok = qc.transpose(1, 0, 2).reshape(S, NH, HD)   # (512, NH, 64)
        q_sc[0:64] = qtok.reshape(NSTRIP, SW, NH, HD).transpose(3, 2, 0, 1)
        q_sc[64] = 1.0
        kt_sc = np.zeros((65, NKV, NSLOT_TOT, 128), np.float32)
        kt_sc[64] = DEAD_BIAS
        v_sc = np.zeros((128, NKV, NSLOT_TOT, 65), np.float32)
        v_sc[:, :, :, 64] = 1.0
        kb = kn[b].reshape(16, 128, NKV, HD)   # [ktile, n, kv, d]
        vb = vn[b].reshape(16, 128, NKV, HD)
        for strip in range(NSTRIP):
            st = strips[strip]
            n_s = NPOS[strip]
            smap = _slot_map(st, n_s, window)
            for s, (kt, bias) in smap.items():
                po = POS_BASE[strip] + s
                kt_sc[0:64, :, po, :] = kb[kt].transpose(2, 1, 0)
                kt_sc[64, :, po, :] = bias
                v_sc[:, :, po, 0:64] = vb[kt]
        xs = x[b, toks, :]
        x_fm32 = np.ascontiguousarray(
            xs.T.reshape(8, 128, S).transpose(1, 0, 2)).astype(np.float32)
        m = dict(q_sc=q_sc, kt_sc=np.ascontiguousarray(kt_sc),
                 v_sc=np.ascontiguousarray(v_sc), x_fm32=x_fm32,
                 wo_sc=wo_sc, rw_sb=rw_sb)
        if masked:
            wm = np.zeros((128, NSLOT_TOT * SW), np.float32)
            for strip in range(NSTRIP):
                st = strips[strip]
                n_s = NPOS[strip]
                smap = _slot_map(st, n_s, window)
                for s, (kt, bias) in smap.items():
                    po = POS_BASE[strip] + s
                    qg = 2 * st * 128 + np.arange(SW)[None, :]
                    kg = kt * 128 + np.arange(128)[:, None]
                    ok = (kg <= qg) & (qg - kg <= window)
                    wm[:, po * SW:(po + 1) * SW] = ok
            m["wmask"] = wm.astype(np.float32)
        maps1b.append(m)
    res1b = run_bass_kernel_spmd(nc1b, maps1b, list(range(NCORES))).results

    # ---------------- routing ----------------
    N = B * T
    logits = np.zeros((N, E_MLP + E_VE), np.float32)
    x2 = np.zeros((N, C), np.float32)
    xfb = np.zeros((N, C), np.float32)
    for c in range(NCORES):
        b = c // 4
        toks = toks_all[c] + b * T
        logits[toks] = res1b[c]["lg_out"].T
        x2c = res1b[c]["x2_out"]                     # [128 p, 8 cc, 512 n]
        x2[toks] = x2c.transpose(2, 1, 0).reshape(S, C)
        xfc = res1b[c]["xfb_out"].astype(np.float32)
        xfb[toks] = xfc.transpose(2, 1, 0).reshape(S, C)

    sparse_w = _route(logits, router_bias)

    idx_list = [np.nonzero(sparse_w[:, e])[0] for e in range(E_MLP)]
    n_list = [len(ix) for ix in idx_list]
    ncap = NCAP0
    while ncap < max(n_list):
        ncap += 512

    key2 = ("2", ncap)
    if key2 not in _prog_cache:
        _prog_cache[key2] = build_2(ncap)
    nc2 = _prog_cache[key2]

    tok_flat = token_ids.reshape(-1)
    maps2 = []
    for c in range(NCORES):
        e = c
        ix = idx_list[e]
        g = np.sqrt(sparse_w[ix, e]).astype(np.float32)
        xg = np.zeros((C, ncap), np.float32)
        xg[:, :len(ix)] = xfb[ix].T * g[None, :]
        xfg = np.ascontiguousarray(
            xg.reshape(8, 128, ncap).transpose(1, 0, 2)).astype(BF)
        wup_m = np.ascontiguousarray(
            w_up[e].T.reshape(8, 128, HID).transpose(1, 0, 2)).astype(BF)
        wdn_dr = np.ascontiguousarray(
            w_down[e].T.reshape(16, 128, C).transpose(1, 0, 2)).astype(BF)
        b = c // 4
        toks = toks_all[c] + b * T
        tids = tok_flat[toks]
        ve0 = (64.0 * sparse_w[toks, E_MLP, None]
               * ve_tables[0][tids]).reshape(QT, 128, C).transpose(1, 0, 2)
        ve1 = (64.0 * sparse_w[toks, E_MLP + 1, None]
               * ve_tables[1][tids]).reshape(QT, 128, C).transpose(1, 0, 2)
        maps2.append(dict(xfg=xfg, wup=wup_m, wdn=wdn_dr,
                          ve0=np.ascontiguousarray(ve0).astype(E4),
                          ve1=np.ascontiguousarray(ve1).astype(E4)))
    res2 = run_bass_kernel_spmd(nc2, maps2, list(range(NCORES))).results

    out = x2.copy()
    for c in range(NCORES):
        b = c // 4
        toks = toks_all[c] + b * T
        veo = res2[c]["ve_out"].astype(np.float32)  # [128, QT, C]
        out[toks] += veo.transpose(1, 0, 2).reshape(S, C) / 64.0
    for e in range(E_MLP):
        n_e = n_list[e]
        if n_e:
            moe = res2[e]["moe_out"].astype(np.float32)  # [128, 8, ncap]
            out[idx_list[e]] += moe[:, :, :n_e].transpose(
                2, 1, 0).reshape(n_e, C)
    return out.reshape(B, T, C).astype(np.float32)


# revision 52
# speedup vs baseline: 1.0488x; 1.0303x over previous
"""TRN2 Bass kernel for nn_BlockMoVaE (attention + MoE/VE routing block).

Self-contained: accepts FULL inputs, shards across 8 NeuronCores, returns
FULL output.

Three SPMD launches with host re-layout between them (host does only data
movement / routing; all FLOPs stay on device).  The router top-2 decision
is discrete, so everything feeding the logits (all of phase 1) runs at
f32r precision; only the post-routing expert MLP uses bf16/fp8.

  L1a  (token-parallel, 512 tokens/core = 2 causally-balanced 256-strips):
       x-rms stats, raw-x f32r QKV projections, token-major rope +
       per-head rmsnorm (the per-token x-norm scalar cancels inside the
       head rmsnorm, so Q/K project raw x; V is scaled by r explicitly).
       Exports token-major q/k/v (f32).
  L1b  (token-parallel attention): host re-lays q/k/v into score-friendly
       f32r layouts.  Causal work is balanced by giving core ci the
       256-token strips {ci, 7-ci} of its batch; the static program
       computes (8, 16) key-128-slots for the two strips; fully-dead
       slots are killed by a rank-1 bias row (-30000) folded into the
       score matmul contraction, diagonal tiles by static affine_selects.
       Softmax denominators ride along as a ones-column of V.  Then
       wo + residual + xf rmsnorm + router logits.
  L2   (expert-parallel MoE): host routes top-2 and gathers tokens per
       expert with sqrt(gate) pre-scaling (relu^2 is 2-homogeneous so the
       gate factors exactly); bf16 up / fp8-DoubleRow down projections;
       VE rows host-gathered, gate-pre-scaled, summed on device.
"""
import numpy as np
import ml_dtypes

import concourse.bass as bass
import concourse.bacc as bacc
import concourse.mybir as mybir
import concourse.tile as tile
from concourse.bass_utils import run_bass_kernel_spmd
from concourse.alu_op_type import AluOpType
from contextlib import ExitStack
from collections import deque

# ---- problem constants (hardcoded per contest rules) ----
B, T, C = 2, 2048, 1024
NH, NKV, HD = 16, 8, 64
E_MLP, E_VE, TOPK = 8, 2, 2
HID = 2048
VOCAB = 50257
EPS = 1e-6
NCORES = 8
S = 512              # tokens per core
QT = 4               # 128-token tiles per core (2 strips of 256)
NSTRIP = 2
SW = 256             # strip width (queries)
NPOS = (8, 16)       # static key-slot count per strip
POS_BASE = (0, 8)    # slot base in kT layout (total 24)
NSLOT_TOT = 24
DEAD_BIAS = -30000.0
NCAP0 = 1024

f32 = mybir.dt.float32
f32r = mybir.dt.float32r
bf16 = mybir.dt.bfloat16
fp8e4 = mybir.dt.float8e4
AF = mybir.ActivationFunctionType
DR = mybir.MatmulPerfMode.DoubleRow
E4 = ml_dtypes.float8_e4m3
BF = ml_dtypes.bfloat16

_prog_cache = {}


def _register_consts(nc, values):
    for value in values:
        key = (f32, float(value))
        if key not in nc.const_aps.aps:
            t = nc.alloc_sbuf_tensor(f"constap-{value}", [128, 1], f32)
            nc.gpsimd.memset(t.ap(), float(value))
            nc.const_aps.aps[key] = t.ap()
    nc.all_engine_barrier()


# --------------------------------------------------------------------------
# L1a: x stats + QKV projection + rope + head-rms (token-major epilogues)
# --------------------------------------------------------------------------
def build_1a():
    nc = bacc.Bacc("TRN2", target_bir_lowering=False, debug=False,
                   num_devices=NCORES)

    x_fm = nc.dram_tensor("x_fm", [128, 8, S], f32r, kind="ExternalInput").ap()
    x_tm = nc.dram_tensor("x_tm", [128, QT, C], f32, kind="ExternalInput").ap()
    wq_t = nc.dram_tensor("wq_t", [128, 8, NH * HD], f32r,
                          kind="ExternalInput").ap()
    wk_t = nc.dram_tensor("wk_t", [128, 8, NKV * HD], f32r,
                          kind="ExternalInput").ap()
    wv_t = nc.dram_tensor("wv_t", [128, 8, NKV * HD], f32r,
                          kind="ExternalInput").ap()
    cos_tm = nc.dram_tensor("cos_tm", [128, QT, 32], f32,
                            kind="ExternalInput").ap()
    sin_tm = nc.dram_tensor("sin_tm", [128, QT, 32], f32,
                            kind="ExternalInput").ap()

    q_out = nc.dram_tensor("q_out", [128, QT, NH * HD], f32,
                           kind="ExternalOutput").ap()
    k_out = nc.dram_tensor("k_out", [128, QT, NKV * HD], f32,
                           kind="ExternalOutput").ap()
    v_out = nc.dram_tensor("v_out", [128, QT, NKV * HD], f32,
                           kind="ExternalOutput").ap()

    _register_consts(nc, [EPS])
    with tile.TileContext(nc) as tc, ExitStack() as est:
        wp = est.enter_context(tc.tile_pool(name="wp", bufs=1))
        work = est.enter_context(tc.tile_pool(name="work", bufs=2))
        outp = est.enter_context(tc.tile_pool(name="outp", bufs=1))
        ps_q = est.enter_context(tc.tile_pool(name="ps_q", bufs=2, space="PSUM"))
        ps_k = est.enter_context(tc.tile_pool(name="ps_k", bufs=2, space="PSUM"))
        ps_v = est.enter_context(tc.tile_pool(name="ps_v", bufs=2, space="PSUM"))

        xf = wp.tile([128, 8, S], f32r, name="xf")
        xt = wp.tile([128, QT, C], f32, name="xt")
        wq = wp.tile([128, 8, NH * HD], f32r, name="wq")
        wk = wp.tile([128, 8, NKV * HD], f32r, name="wk")
        wv = wp.tile([128, 8, NKV * HD], f32r, name="wv")
        cs = wp.tile([128, QT, 32], f32, name="cs")
        sn = wp.tile([128, QT, 32], f32, name="sn")
        for cc in range(8):
            nc.sync.dma_start(xf[:, cc, :], x_fm[:, cc, :])
            nc.sync.dma_start(wq[:, cc, :], wq_t[:, cc, :])
        for cc in range(8):
            nc.sync.dma_start(wk[:, cc, :], wk_t[:, cc, :])
            nc.sync.dma_start(wv[:, cc, :], wv_t[:, cc, :])
        nc.sync.dma_start(cs[:], cos_tm[:])
        nc.sync.dma_start(sn[:], sin_tm[:])
        nc.sync.dma_start(xt[:], x_tm[:])

        qe = outp.tile([128, QT, NH * HD], f32, name="qe")
        ke = outp.tile([128, QT, NKV * HD], f32, name="ke")
        ve = outp.tile([128, QT, NKV * HD], f32, name="ve")

        def rope_norm(ps, nh, t, out_tile):
            """Token-major rope + per-head rmsnorm from psum [128, nh*64]."""
            qs = work.tile([128, nh, HD], f32, tag=f"qs{nh}", name="qs")
            nc.scalar.copy(qs[:], ps[:].rearrange("p (h d) -> p h d", d=HD))
            cosb = cs[:, t:t + 1, :].broadcast_to([128, nh, 32])
            sinb = sn[:, t:t + 1, :].broadcast_to([128, nh, 32])
            rp = work.tile([128, nh, HD], f32, tag=f"rp{nh}", name="rp")
            a = work.tile([128, nh, 32], f32, tag=f"ra{nh}", name="ra")
            b = work.tile([128, nh, 32], f32, tag=f"rb{nh}", name="rb")
            c2 = work.tile([128, nh, 32], f32, tag=f"rc{nh}", name="rc")
            d2 = work.tile([128, nh, 32], f32, tag=f"rd{nh}", name="rd")
            nc.vector.tensor_mul(a[:], qs[:, :, 0:32], cosb)
            nc.vector.tensor_mul(b[:], qs[:, :, 32:64], sinb)
            nc.gpsimd.tensor_mul(c2[:], qs[:, :, 32:64], cosb)
            nc.gpsimd.tensor_mul(d2[:], qs[:, :, 0:32], sinb)
            nc.gpsimd.tensor_add(rp[:, :, 0:32], a[:], b[:])
            nc.vector.tensor_sub(rp[:, :, 32:64], c2[:], d2[:])
            sq = work.tile([128, nh, HD], f32, tag=f"sq{nh}", name="sq")
            nc.scalar.activation(sq[:], rp[:], AF.Square)
            hs = work.tile([128, nh, 1], f32, tag=f"hs{nh}", name="hs")
            nc.vector.tensor_reduce(out=hs[:], in_=sq[:], op=AluOpType.add,
                                    axis=mybir.AxisListType.X)
            sh = work.tile([128, nh, 1], f32, tag=f"sh{nh}", name="sh")
            nc.scalar.activation(sh[:], hs[:], AF.Sqrt, bias=EPS,
                                 scale=1.0 / HD)
            rh = work.tile([128, nh, 1], f32, tag=f"rh{nh}", name="rh")
            with nc.allow_low_precision(reason="head rms recip"):
                nc.vector.reciprocal(rh[:], sh[:])
            nc.vector.tensor_mul(
                out_tile[:].rearrange("p (h d) -> p h d", d=HD),
                rp[:], rh[:].broadcast_to([128, nh, HD]))

        for t in range(QT):
            # per-token inv-rms of x (V only; cancels inside Q/K head-rms)
            xsq = work.tile([128, C], f32, tag="xsq", name="xsq")
            nc.scalar.activation(xsq[:], xt[:, t, :], AF.Square)
            ssq = work.tile([128, 1], f32, tag="ssq", name="ssq")
            nc.vector.tensor_reduce(out=ssq[:], in_=xsq[:], op=AluOpType.add,
                                    axis=mybir.AxisListType.XYZW)
            sx = work.tile([128, 1], f32, tag="sx", name="sx")
            nc.scalar.activation(sx[:], ssq[:], AF.Sqrt, bias=EPS,
                                 scale=1.0 / C)
            r = work.tile([128, 1], f32, tag="r", name="r")
            with nc.allow_low_precision(reason="x rms recip"):
                nc.vector.reciprocal(r[:], sx[:])

            q_ps = ps_q.tile([128, NH * HD], f32, name="q_ps")
            k_ps = ps_k.tile([128, NKV * HD], f32, name="k_ps")
            v_ps = ps_v.tile([128, NKV * HD], f32, name="v_ps")
            for half in range(2):
                hsl = bass.ts(half, NH * HD // 2)
                for cc in range(8):
                    nc.tensor.matmul(q_ps[:, hsl],
                                     xf[:, cc, bass.ts(t, 128)],
                                     wq[:, cc, hsl],
                                     start=(cc == 0), stop=(cc == 7))
            for cc in range(8):
                nc.tensor.matmul(k_ps[:], xf[:, cc, bass.ts(t, 128)],
                                 wk[:, cc, :], start=(cc == 0), stop=(cc == 7))
            for cc in range(8):
                nc.tensor.matmul(v_ps[:], xf[:, cc, bass.ts(t, 128)],
                                 wv[:, cc, :], start=(cc == 0), stop=(cc == 7))

            rope_norm(q_ps, NH, t, qe[:, t, :])
            nc.sync.dma_start(q_out[:, t, :], qe[:, t, :])
            rope_norm(k_ps, NKV, t, ke[:, t, :])
            nc.sync.dma_start(k_out[:, t, :], ke[:, t, :])
            nc.vector.tensor_scalar_mul(ve[:, t, :], v_ps[:], r[:])
            nc.sync.dma_start(v_out[:, t, :], ve[:, t, :])

    nc.compile()
    return nc


# --------------------------------------------------------------------------
# L1b: attention + wo + residual + xf rmsnorm + router logits
# --------------------------------------------------------------------------
def build_1b(masked: bool):
    nc = bacc.Bacc("TRN2", target_bir_lowering=False, debug=False,
                   num_devices=NCORES)

    # q rows 0..63 = head dims, row 64 = 1.0 (rank-1 bias carrier)
    q_sc = nc.dram_tensor("q_sc", [65, NH, NSTRIP, SW], f32r,
                          kind="ExternalInput").ap()
    kt_sc = nc.dram_tensor("kt_sc", [65, NKV, NSLOT_TOT, 128], f32r,
                           kind="ExternalInput").ap()
    # v columns 0..63 = v dims, col 64 = 1.0 (softmax denominator)
    v_sc = nc.dram_tensor("v_sc", [128, NKV, NSLOT_TOT, 65], f32r,
                          kind="ExternalInput").ap()
    x_fm32 = nc.dram_tensor("x_fm32", [128, 8, S], f32,
                            kind="ExternalInput").ap()
    wo_sc = nc.dram_tensor("wo_sc", [128, 8, C], f32r,
                           kind="ExternalInput").ap()
    rw_sb = nc.dram_tensor("rw_sb", [128, 8, E_MLP + E_VE], f32,
                           kind="ExternalInput").ap()
    if masked:
        wmask = nc.dram_tensor("wmask", [128, NSLOT_TOT * SW], f32,
                               kind="ExternalInput").ap()

    x2_out = nc.dram_tensor("x2_out", [128, 8, S], f32,
                            kind="ExternalOutput").ap()
    xfb_out = nc.dram_tensor("xfb_out", [128, 8, S], bf16,
                             kind="ExternalOutput").ap()
    lg_out = nc.dram_tensor("lg_out", [E_MLP + E_VE, S], f32,
                            kind="ExternalOutput").ap()

    _register_consts(nc, [EPS])
    with tile.TileContext(nc) as tc, ExitStack() as est:
        wp = est.enter_context(tc.tile_pool(name="wp", bufs=1))
        ytp = est.enter_context(tc.tile_pool(name="ytp", bufs=1))

        yTp = ytp.tile([128, NH // 2, S], f32r, name="yTp")
        yTo = ytp.tile([64, NH // 2, S], f32r, name="yTo")

        with tc.tile_pool(name="ps_sc", bufs=2, space="PSUM") as ps_sc, \
             tc.tile_pool(name="ps_yv", bufs=1, space="PSUM") as ps_yv, \
             tc.tile_pool(name="ps_bc", bufs=1, space="PSUM") as ps_bc, \
             tc.tile_pool(name="attp", bufs=1) as attp, \
             tc.tile_pool(name="kvs", bufs=2) as kvs, \
             tc.tile_pool(name="ptp", bufs=9) as ptp, \
             tc.tile_pool(name="ivp", bufs=2) as ivp:
            q_t = attp.tile([65, NH, NSTRIP, SW], f32r, name="q_t")
            ones64f = attp.tile([1, 64], f32, name="ones64f")
            nc.vector.memset(ones64f[:], 1.0)
            ones64 = attp.tile([1, 64], f32r, name="ones64")
            nc.scalar.copy(ones64[:], ones64f[:])
            if masked:
                wm_t = attp.tile([128, NSLOT_TOT * SW], f32, name="wm_t")
                nc.sync.dma_start(wm_t[:], wmask[:])

            # stream q/kt/v per (strip, kv-pair); each slice loaded once
            kv_tiles = {}
            for strip in range(NSTRIP):
                n_s = NPOS[strip]
                for kp in range(4):
                    nc.sync.dma_start(
                        q_t[:, 4 * kp:4 * kp + 4, strip, :],
                        q_sc[:, 4 * kp:4 * kp + 4, strip, :])
                    kt = kvs.tile([65, 2, n_s, 128], f32r,
                                  tag=f"kt{strip}", name=f"kt{strip}_{kp}")
                    sl = slice(POS_BASE[strip], POS_BASE[strip] + n_s)
                    nc.sync.dma_start(kt[:], kt_sc[:, 2 * kp:2 * kp + 2, sl, :])
                    vt = kvs.tile([128, 2, n_s, 65], f32r,
                                  tag=f"vt{strip}", name=f"vt{strip}_{kp}")
                    nc.sync.dma_start(vt[:], v_sc[:, 2 * kp:2 * kp + 2, sl, :])
                    kv_tiles[(strip, kp)] = (kt, vt)
            x_t = wp.tile([128, 8, S], f32, name="x_t")
            rw_t = wp.tile([128, 8, E_MLP + E_VE], f32, name="rw_t")
            nc.sync.dma_start(x_t[:], x_fm32[:])
            nc.sync.dma_start(rw_t[:], rw_sb[:])

            def emit_scores(strip, hg):
                """Scores + exp + mask for the 4 heads of kv-pair hg."""
                n_s = NPOS[strip]
                kt, _ = kv_tiles[(strip, hg)]
                chunks = [(c0, min(4, n_s - c0)) for c0 in range(0, n_s, 4)]
                pts = []
                for hi in range(4):
                    h = 4 * hg + hi
                    kvl = hi // 2          # kv head within the pair
                    pt_chunks = []
                    for c0, cn in chunks:
                        sc = ps_sc.tile([128, 4 * SW], f32, tag="sc",
                                        name="sc")
                        for s in range(cn):
                            nc.tensor.matmul(
                                sc[:, bass.ts(s, SW)],
                                kt[:, kvl, c0 + s, :],
                                q_t[:, h, strip, :],
                                start=True, stop=True)
                        pt = ptp.tile([128, 4 * SW], f32r, tag="pt",
                                      name=f"pt{strip}_{h}_{c0}")
                        nc.scalar.activation(pt[:, 0:cn * SW],
                                             sc[:, 0:cn * SW],
                                             AF.Exp, scale=0.125)
                        if masked:
                            base = (POS_BASE[strip] + c0) * SW
                            nc.vector.tensor_mul(
                                pt[:, 0:cn * SW], pt[:, 0:cn * SW],
                                wm_t[:, base:base + cn * SW])
                        else:
                            if c0 + cn == n_s:
                                # main diagonal tile (last slot): q-half 0
                                # is fully future -> zero; q-half 1: k <= q
                                off = (cn - 1) * SW
                                zsl = pt[:, off:off + 128]
                                nc.gpsimd.affine_select(
                                    zsl, zsl, pattern=[[1, 128]], base=-128,
                                    channel_multiplier=-1,
                                    compare_op=AluOpType.is_ge, fill=0.0)
                                dsl = pt[:, off + 128:off + 256]
                                nc.gpsimd.affine_select(
                                    dsl, dsl, pattern=[[1, 128]], base=0,
                                    channel_multiplier=-1,
                                    compare_op=AluOpType.is_ge, fill=0.0)
                                # sub-diagonal (slot n_s-2): q-half 0 k<=q
                                if cn >= 2:
                                    ssl = pt[:, off - SW:off - 128]
                                    nc.gpsimd.affine_select(
                                        ssl, ssl, pattern=[[1, 128]], base=0,
                                        channel_multiplier=-1,
                                        compare_op=AluOpType.is_ge, fill=0.0)
                        pt_chunks.append(pt)
                    pts.append(pt_chunks)
                return pts, chunks

            def emit_yv(strip, hg, pts, chunks):
                """p@v accumulate (ones col 64 -> den row 64) + normalize."""
                n_s = NPOS[strip]
                _, vt = kv_tiles[(strip, hg)]
                yv_ps = ps_yv.tile([65, 4 * SW], f32, tag="yv",
                                   name=f"yv{strip}_{hg}")
                for hi in range(4):
                    kvl = hi // 2
                    for (c0, cn), pt in zip(chunks, pts[hi]):
                        for s in range(cn):
                            nc.tensor.matmul(
                                yv_ps[:, bass.ts(hi, SW)],
                                vt[:, kvl, c0 + s, :],
                                pt[:, bass.ts(s, SW)],
                                start=(c0 + s == 0),
                                stop=(c0 + s == n_s - 1))
                iv = ivp.tile([1, 4 * SW], f32r, tag="iv", name="iv")
                with nc.allow_low_precision(reason="softmax recip"):
                    nc.vector.reciprocal(iv[:], yv_ps[64:65, :])
                bc_ps = ps_bc.tile([64, 4 * SW], f32, tag="bc", name="bc")
                for hi in range(4):
                    nc.tensor.matmul(bc_ps[:, bass.ts(hi, SW)],
                                     ones64[:], iv[0:1, bass.ts(hi, SW)],
                                     start=True, stop=True)
                bc_sb = ivp.tile([64, 4 * SW], f32, tag="bcs", name="bcs")
                nc.vector.tensor_copy(bc_sb[:], bc_ps[:])
                yv4 = yv_ps[0:64, :].rearrange("p (h n) -> p h n", h=4)
                bc4 = bc_sb[:].rearrange("p (h n) -> p h n", h=4)
                ssl = bass.ts(strip, SW)
                # even heads (hi 0,2) -> chunks 2hg..2hg+1 rows 0:64
                nc.vector.tensor_mul(
                    yTp[0:64, 2 * hg:2 * hg + 2, ssl],
                    yv4[:, 0:4:2, :], bc4[:, 0:4:2, :])
                # odd heads -> staging, then partition-shift DMA
                nc.vector.tensor_mul(
                    yTo[:, 2 * hg:2 * hg + 2, ssl],
                    yv4[:, 1:4:2, :], bc4[:, 1:4:2, :])
                nc.sync.dma_start(yTp[64:128, 2 * hg:2 * hg + 2, ssl],
                                  yTo[:, 2 * hg:2 * hg + 2, ssl])

            pending = deque()
            LAG = 2
            for strip in range(NSTRIP):
                for hg in range(4):
                    pts, chunks = emit_scores(strip, hg)
                    pending.append((strip, hg, pts, chunks))
                    if len(pending) > LAG:
                        emit_yv(*pending.popleft())
            while pending:
                emit_yv(*pending.popleft())

        # ---- wo + residual + xf rmsnorm + router ----
        with tc.tile_pool(name="ps_at", bufs=2, space="PSUM") as ps_at, \
             tc.tile_pool(name="ps_row", bufs=2, space="PSUM") as ps_row, \
             tc.tile_pool(name="ps_bcf", bufs=1, space="PSUM") as ps_bcf, \
             tc.tile_pool(name="tl", bufs=2) as tl, \
             tc.tile_pool(name="x2p", bufs=1) as x2p:
            ones_f = tl.tile([128, 1], f32, tag="onesf", name="ones_f", bufs=1)
            nc.vector.memset(ones_f[:], 1.0)
            ones_col = tl.tile([128, 1], f32r, tag="onesc", name="ones_col",
                               bufs=1)
            nc.scalar.copy(ones_col[:], ones_f[:])
            ones_rf = tl.tile([1, 128], f32, tag="onesrf", name="ones_rf",
                              bufs=1)
            nc.vector.memset(ones_rf[:], 1.0)
            ones_row = tl.tile([1, 128], f32r, tag="onesr", name="ones_row",
                               bufs=1)
            nc.scalar.copy(ones_row[:], ones_rf[:])

            x2w = x2p.tile([128, 8, S], f32, name="x2w")
            ssq_f = ps_bcf.tile([1, S], f32, tag="ssqf", name="ssq_f")
            rt_ps = ps_row.tile([E_MLP + E_VE, S], f32, tag="rt", name="rt_ps")
            wo_tiles = []
            for co in range(8):
                wo_t = tl.tile([128, 8, 128], f32r, tag="wo",
                               name=f"wo{co}", bufs=5)
                nc.sync.dma_start(wo_t[:], wo_sc[:, :, bass.ts(co, 128)])
                wo_tiles.append(wo_t)
            sqfs = []
            for co in range(8):
                at_ps = ps_at.tile([128, S], f32, tag="at", name="at_ps")
                for cc in range(8):
                    nc.tensor.matmul(
                        at_ps[:], wo_tiles[co][:, cc, :],
                        yTp[:, cc, :],
                        start=(cc == 0), stop=(cc == 7))
                nc.vector.tensor_add(x2w[:, co, :], at_ps[:], x_t[:, co, :])
                nc.sync.dma_start(x2_out[:, co, :], x2w[:, co, :])
                sqf = tl.tile([128, S], f32r, tag="sqf", name=f"sqf{co}",
                              bufs=8)
                nc.scalar.activation(sqf[:], x2w[:, co, :], AF.Square)
                sqfs.append(sqf)
            for co in range(8):
                nc.tensor.matmul(ssq_f[:], ones_col[:], sqfs[co][:],
                                 start=(co == 0), stop=(co == 7))
                nc.tensor.matmul(rt_ps[:], rw_t[:, co, :], x2w[:, co, :],
                                 start=(co == 0), stop=(co == 7))

            srow = tl.tile([1, S], f32, tag="srow", name="srow", bufs=1)
            rrow = tl.tile([1, S], f32r, tag="rrow", name="rrow", bufs=1)
            bcf_sb = tl.tile([128, S], f32, tag="bcfs", name="bcf_sb", bufs=1)
            xfb = x2p.tile([128, 8, S], bf16, name="xfb")
            lg = tl.tile([E_MLP + E_VE, S], f32, tag="lg", name="lg", bufs=1)
            for hf in range(2):
                fsl = bass.ts(hf, SW)
                nc.scalar.activation(srow[0:1, fsl], ssq_f[0:1, fsl],
                                     AF.Sqrt, bias=EPS, scale=1.0 / C)
                with nc.allow_low_precision(reason="f32r rms bcast rows"):
                    nc.vector.reciprocal(rrow[0:1, fsl], srow[0:1, fsl])
                bcf_ps = ps_row.tile([128, SW], f32, tag="bcf", name="bcf_ps")
                nc.tensor.matmul(bcf_ps[:], ones_row[:], rrow[0:1, fsl],
                                 start=True, stop=True)
                nc.vector.tensor_copy(bcf_sb[:, fsl], bcf_ps[:])
                nc.vector.tensor_mul(lg[:, fsl], rt_ps[:, fsl],
                                     bcf_sb[0:E_MLP + E_VE, fsl])
                for co in range(8):
                    eng = nc.vector if co % 2 == 0 else nc.gpsimd
                    eng.tensor_mul(xfb[:, co, fsl], x2w[:, co, fsl],
                                   bcf_sb[:, fsl])
                    nc.sync.dma_start(xfb_out[:, co, fsl], xfb[:, co, fsl])
            nc.sync.dma_start(lg_out[:], lg[:])

    nc.compile()
    return nc


# --------------------------------------------------------------------------
# L2: expert MLP (bf16 up, fp8-DR down, gate pre-folded) + VE sum
# --------------------------------------------------------------------------
def build_2(ncap: int):
    """Expert MLP via 3-pass fp8 hi/lo DoubleRow: w*x ~ wh*xh + wh*xl + wl*xh
    (the dropped wl*xl term is ~1e-3 relative).  ~bf16 precision at half
    the PE cycles and smaller weight DMA."""
    nc = bacc.Bacc("TRN2", target_bir_lowering=False, debug=False,
                   num_devices=NCORES)
    NT = ncap // 512

    x_hi = nc.dram_tensor("x_hi", [128, 8, ncap], fp8e4,
                          kind="ExternalInput").ap()
    x_lo = nc.dram_tensor("x_lo", [128, 8, ncap], fp8e4,
                          kind="ExternalInput").ap()
    up_hi = nc.dram_tensor("up_hi", [128, 8, HID], fp8e4,
                           kind="ExternalInput").ap()
    up_lo = nc.dram_tensor("up_lo", [128, 8, HID], fp8e4,
                           kind="ExternalInput").ap()
    dn_hi = nc.dram_tensor("dn_hi", [128, 16, C], fp8e4,
                           kind="ExternalInput").ap()
    dn_lo = nc.dram_tensor("dn_lo", [128, 16, C], fp8e4,
                           kind="ExternalInput").ap()
    ve0 = nc.dram_tensor("ve0", [128, QT, C], fp8e4, kind="ExternalInput").ap()
    ve1 = nc.dram_tensor("ve1", [128, QT, C], fp8e4, kind="ExternalInput").ap()

    moe_out = nc.dram_tensor("moe_out", [128, 8, ncap], bf16,
                             kind="ExternalOutput").ap()
    ve_out = nc.dram_tensor("ve_out", [128, QT, C], bf16,
                            kind="ExternalOutput").ap()

    with tile.TileContext(nc) as tc, ExitStack() as est:
        wp = est.enter_context(tc.tile_pool(name="wp", bufs=1))
        hp = est.enter_context(tc.tile_pool(name="hp", bufs=2))
        op = est.enter_context(tc.tile_pool(name="op", bufs=3))
        ps_h = est.enter_context(tc.tile_pool(name="ps_h", bufs=3, space="PSUM"))
        ps_o = est.enter_context(tc.tile_pool(name="ps_o", bufs=3, space="PSUM"))

        xh_t = wp.tile([128, 8, ncap], fp8e4, name="xh_t")
        xl_t = wp.tile([128, 8, ncap], fp8e4, name="xl_t")
        uh_t = wp.tile([128, 8, HID], fp8e4, name="uh_t")
        ul_t = wp.tile([128, 8, HID], fp8e4, name="ul_t")
        dh_t = wp.tile([128, 16, C], fp8e4, name="dh_t")
        dl_t = wp.tile([128, 16, C], fp8e4, name="dl_t")
        v0_t = wp.tile([128, QT, C], fp8e4, name="v0_t")
        v1_t = wp.tile([128, QT, C], fp8e4, name="v1_t")
        for cc in range(8):
            nc.sync.dma_start(xh_t[:, cc, :], x_hi[:, cc, :])
            nc.sync.dma_start(uh_t[:, cc, :], up_hi[:, cc, :])
        nc.sync.dma_start(xl_t[:], x_lo[:])
        nc.sync.dma_start(ul_t[:], up_lo[:])
        nc.sync.dma_start(dh_t[:], dn_hi[:])
        nc.sync.dma_start(dl_t[:], dn_lo[:])
        nc.sync.dma_start(v0_t[:], ve0[:])
        nc.sync.dma_start(v1_t[:], ve1[:])

        for nt in range(NT):
            csl = bass.ts(nt, 512)
            h_hi = hp.tile([128, 16, 512], fp8e4, tag="hh", name=f"hh{nt}")
            h_lo = hp.tile([128, 16, 512], fp8e4, tag="hl", name=f"hl{nt}")
            for hc in range(16):
                h_ps = ps_h.tile([128, 512], f32, tag="hps", name="h_ps")
                hsl = bass.ts(hc, 128)
                passes = [(uh_t, xh_t), (uh_t, xl_t), (ul_t, xh_t)]
                for pi, (wt, xt_) in enumerate(passes):
                    for cc in range(4):
                        nc.tensor.matmul(
                            h_ps[:], wt[:, 2 * cc:2 * cc + 2, hsl],
                            xt_[:, 2 * cc:2 * cc + 2, csl],
                            start=(pi == 0 and cc == 0),
                            stop=(pi == 2 and cc == 3), perf_mode=DR)
                hr = op.tile([128, 512], f32, tag="hr", name="hr")
                nc.scalar.activation(hr[:], h_ps[:], AF.Relu)
                hsq = op.tile([128, 512], f32, tag="hsq", name="hsq")
                nc.vector.tensor_mul(hsq[:], hr[:], hr[:])
                eng = nc.vector if hc % 2 == 0 else nc.gpsimd
                eng.tensor_copy(h_hi[:, hc, :], hsq[:])
                eng2 = nc.gpsimd if hc % 2 == 0 else nc.vector
                eng2.tensor_sub(h_lo[:, hc, :], hsq[:], h_hi[:, hc, :])
            for co in range(8):
                o_ps = ps_o.tile([128, 512], f32, tag="ops", name="o_ps")
                osl = bass.ts(co, 128)
                passes = [(dh_t, h_hi), (dh_t, h_lo), (dl_t, h_hi)]
                for pi, (wt, ht_) in enumerate(passes):
                    for hh in range(8):
                        nc.tensor.matmul(
                            o_ps[:], wt[:, 2 * hh:2 * hh + 2, osl],
                            ht_[:, 2 * hh:2 * hh + 2, :],
                            start=(pi == 0 and hh == 0),
                            stop=(pi == 2 and hh == 7), perf_mode=DR)
                ot = op.tile([128, 512], bf16, tag="ot", name="ot")
                nc.scalar.copy(ot[:], o_ps[:])
                nc.sync.dma_start(moe_out[:, co, csl], ot[:])

        vo = op.tile([128, QT, C], bf16, tag="vo", name="vo", bufs=1)
        for t in range(QT):
            eng = nc.vector if t % 2 == 0 else nc.gpsimd
            eng.tensor_add(vo[:, t, :], v0_t[:, t, :], v1_t[:, t, :])
        nc.sync.dma_start(ve_out[:], vo[:])

    nc.compile()
    return nc


# --------------------------------------------------------------------------
# Host orchestration
# --------------------------------------------------------------------------
def _qtiles(ci):
    # strip A = 256-tile ci (128-tiles 2ci, 2ci+1), strip B = 256-tile 7-ci
    sa, sb = ci, 7 - ci
    return [2 * sa, 2 * sa + 1, 2 * sb, 2 * sb + 1]


def _slot_map(st, n_s, window):
    """slot -> (key 128-tile, bias).  Diagonal tile (2st+1) at slot n_s-1,
    sub-diagonal (2st) at n_s-2, other causally-alive tiles from slot 0."""
    alive = [kt for kt in range(2 * st)
             if window >= T or 128 * (2 * st + 1 - kt) - 127 <= window]
    m = {}
    for si, kt in enumerate(alive):
        m[si] = (kt, 0.0)
    m[n_s - 2] = (2 * st, 0.0)
    m[n_s - 1] = (2 * st + 1, 0.0)
    return m


def _route(logits, router_bias):
    sig = (1.0 / (1.0 + np.exp(-logits.astype(np.float32)))).astype(np.float32)
    sel = sig + router_bias[None, :].astype(np.float32)
    idx = np.argsort(-sel, axis=1, kind="stable")[:, :TOPK]
    tw = np.take_along_axis(sig, idx, axis=1)
    tw = tw / tw.sum(axis=1, keepdims=True)
    N = logits.shape[0]
    sparse_w = np.zeros((N, E_MLP + E_VE), np.float32)
    np.put_along_axis(sparse_w, idx, tw, axis=1)
    return sparse_w


def kernel(**inputs):
    x = np.asarray(inputs["x"], np.float32)
    token_ids = np.asarray(inputs["token_ids"])
    cos = np.asarray(inputs["cos"], np.float32)
    sin = np.asarray(inputs["sin"], np.float32)
    window = int(np.asarray(inputs["window_size"]))
    wq, wk, wv, wo = (np.asarray(inputs[k], np.float32)
                      for k in ("wq", "wk", "wv", "wo"))
    w_up = np.asarray(inputs["w_up"], np.float32)
    w_down = np.asarray(inputs["w_down"], np.float32)
    router_w = np.asarray(inputs["router_w"], np.float32)
    router_bias = np.asarray(inputs["router_bias"], np.float32)
    ve_tables = np.asarray(inputs["ve_tables"], np.float32)

    cosf = cos[0, :, 0, :]      # (T, 32)
    sinf = sin[0, :, 0, :]

    # ---------------- L1a ----------------
    if "1a" not in _prog_cache:
        _prog_cache["1a"] = build_1a()
    nc1a = _prog_cache["1a"]

    def mov_w(w):  # (M, C) -> [128, 8, M] with [p, cc, m] = w[m, 128cc+p]
        M = w.shape[0]
        return np.ascontiguousarray(
            w.T.reshape(8, 128, M).transpose(1, 0, 2)).astype(np.float32)

    wq_t, wk_t, wv_t = mov_w(wq), mov_w(wk), mov_w(wv)
    toks_all, maps1a = [], []
    for c in range(NCORES):
        b, ci = c // 4, c % 4
        qts = _qtiles(ci)
        toks = np.concatenate([np.arange(qt * 128, qt * 128 + 128)
                               for qt in qts])
        toks_all.append(toks)
        xs = x[b, toks, :]                      # (512, 1024)
        x_fm = np.ascontiguousarray(
            xs.T.reshape(8, 128, S).transpose(1, 0, 2)).astype(np.float32)
        x_tm = np.ascontiguousarray(
            xs.reshape(QT, 128, C).transpose(1, 0, 2)).astype(np.float32)
        cos_tm = np.ascontiguousarray(
            cosf[toks].reshape(QT, 128, 32).transpose(1, 0, 2)
        ).astype(np.float32)
        sin_tm = np.ascontiguousarray(
            sinf[toks].reshape(QT, 128, 32).transpose(1, 0, 2)
        ).astype(np.float32)
        maps1a.append(dict(x_fm=x_fm, x_tm=x_tm, wq_t=wq_t, wk_t=wk_t,
                           wv_t=wv_t, cos_tm=cos_tm, sin_tm=sin_tm))
    res1a = run_bass_kernel_spmd(nc1a, maps1a, list(range(NCORES))).results

    kn = np.zeros((B, T, NKV * HD), np.float32)
    vn = np.zeros((B, T, NKV * HD), np.float32)
    qn = []
    for c in range(NCORES):
        b = c // 4
        toks = toks_all[c]
        kc = res1a[c]["k_out"].astype(np.float32)   # [128, QT, 512]
        vc = res1a[c]["v_out"].astype(np.float32)
        kn[b, toks.reshape(QT, 128)] = kc.transpose(1, 0, 2)
        vn[b, toks.reshape(QT, 128)] = vc.transpose(1, 0, 2)
        qn.append(res1a[c]["q_out"].astype(np.float32))  # [128, QT, 1024]

    # ---------------- L1b ----------------
    masked = window < T
    key1b = ("1b", masked)
    if key1b not in _prog_cache:
        _prog_cache[key1b] = build_1b(masked)
    nc1b = _prog_cache[key1b]

    wo_sc = np.ascontiguousarray(
        wo.T.reshape(8, 128, C).transpose(1, 0, 2)).astype(np.float32)
    rw_sb = np.ascontiguousarray(
        router_w.T.reshape(8, 128, E_MLP + E_VE).transpose(1, 0, 2)
    ).astype(np.float32)

    maps1b = []
    for c in range(NCORES):
        b, ci = c // 4, c % 4
        strips = [ci, 7 - ci]          # 256-token strip indices
        toks = toks_all[c]
        q_sc = np.zeros((65, NH, NSTRIP, SW), np.float32)
        qc = qn[c]                     # [128, QT, 1024] token-major
        qtok = qc.transpose(1, 0, 2).reshape(S, NH, HD)   # (512, NH, 64)
        q_sc[0:64] = qtok.reshape(NSTRIP, SW, NH, HD).transpose(3, 2, 0, 1)
        q_sc[64] = 1.0
        kt_sc = np.zeros((65, NKV, NSLOT_TOT, 128), np.float32)
        kt_sc[64] = DEAD_BIAS
        v_sc = np.zeros((128, NKV, NSLOT_TOT, 65), np.float32)
        v_sc[:, :, :, 64] = 1.0
        kb = kn[b].reshape(16, 128, NKV, HD)   # [ktile, n, kv, d]
        vb = vn[b].reshape(16, 128, NKV, HD)
        for strip in range(NSTRIP):
            st = strips[strip]
            n_s = NPOS[strip]
            smap = _slot_map(st, n_s, window)
            for s, (kt, bias) in smap.items():
                po = POS_BASE[strip] + s
                kt_sc[0:64, :, po, :] = kb[kt].transpose(2, 1, 0)
                kt_sc[64, :, po, :] = bias
                v_sc[:, :, po, 0:64] = vb[kt]
        xs = x[b, toks, :]
        x_fm32 = np.ascontiguousarray(
            xs.T.reshape(8, 128, S).transpose(1, 0, 2)).astype(np.float32)
        m = dict(q_sc=q_sc, kt_sc=np.ascontiguousarray(kt_sc),
                 v_sc=np.ascontiguousarray(v_sc), x_fm32=x_fm32,
                 wo_sc=wo_sc, rw_sb=rw_sb)
        if masked:
            wm = np.zeros((128, NSLOT_TOT * SW), np.float32)
            for strip in range(NSTRIP):
                st = strips[strip]
                n_s = NPOS[strip]
                smap = _slot_map(st, n_s, window)
                for s, (kt, bias) in smap.items():
                    po = POS_BASE[strip] + s
                    qg = 2 * st * 128 + np.arange(SW)[None, :]
                    kg = kt * 128 + np.arange(128)[:, None]
                    ok = (kg <= qg) & (qg - kg <= window)
                    wm[:, po * SW:(po + 1) * SW] = ok
            m["wmask"] = wm.astype(np.float32)
        maps1b.append(m)
    res1b = run_bass_kernel_spmd(nc1b, maps1b, list(range(NCORES))).results

    # ---------------- routing ----------------
    N = B * T
    logits = np.zeros((N, E_MLP + E_VE), np.float32)
    x2 = np.zeros((N, C), np.float32)
    xfb = np.zeros((N, C), np.float32)
    for c in range(NCORES):
        b = c // 4
        toks = toks_all[c] + b * T
        logits[toks] = res1b[c]["lg_out"].T
        x2c = res1b[c]["x2_out"]                     # [128 p, 8 cc, 512 n]
        x2[toks] = x2c.transpose(2, 1, 0).reshape(S, C)
        xfc = res1b[c]["xfb_out"].astype(np.float32)
        xfb[toks] = xfc.transpose(2, 1, 0).reshape(S, C)

    sparse_w = _route(logits, router_bias)

    idx_list = [np.nonzero(sparse_w[:, e])[0] for e in range(E_MLP)]
    n_list = [len(ix) for ix in idx_list]
    ncap = NCAP0
    while ncap < max(n_list):
        ncap += 512

    key2 = ("2", ncap)
    if key2 not in _prog_cache:
        _prog_cache[key2] = build_2(ncap)
    nc2 = _prog_cache[key2]

    tok_flat = token_ids.reshape(-1)
    maps2 = []
    for c in range(NCORES):
        e = c
        ix = idx_list[e]
        g = np.sqrt(sparse_w[ix, e]).astype(np.float32)
        xg = np.zeros((C, ncap), np.float32)
        xg[:, :len(ix)] = xfb[ix].T * g[None, :]
        def hilo(a):
            hi = a.astype(E4)
            lo = (a - hi.astype(np.float32)).astype(E4)
            return hi, lo

        xfg = np.ascontiguousarray(
            xg.reshape(8, 128, ncap).transpose(1, 0, 2))
        x_hi, x_lo = hilo(xfg)
        up_hi, up_lo = hilo(np.ascontiguousarray(
            w_up[e].T.reshape(8, 128, HID).transpose(1, 0, 2)))
        dn_hi, dn_lo = hilo(np.ascontiguousarray(
            w_down[e].T.reshape(16, 128, C).transpose(1, 0, 2)))
        b = c // 4
        toks = toks_all[c] + b * T
        tids = tok_flat[toks]
        ve0 = (64.0 * sparse_w[toks, E_MLP, None]
               * ve_tables[0][tids]).reshape(QT, 128, C).transpose(1, 0, 2)
        ve1 = (64.0 * sparse_w[toks, E_MLP + 1, None]
               * ve_tables[1][tids]).reshape(QT, 128, C).transpose(1, 0, 2)
        maps2.append(dict(x_hi=x_hi, x_lo=x_lo, up_hi=up_hi, up_lo=up_lo,
                          dn_hi=dn_hi, dn_lo=dn_lo,
                          ve0=np.ascontiguousarray(ve0).astype(E4),
                          ve1=np.ascontiguousarray(ve1).astype(E4)))
    res2 = run_bass_kernel_spmd(nc2, maps2, list(range(NCORES))).results

    out = x2.copy()
    for c in range(NCORES):
        b = c // 4
        toks = toks_all[c] + b * T
        veo = res2[c]["ve_out"].astype(np.float32)  # [128, QT, C]
        out[toks] += veo.transpose(1, 0, 2).reshape(S, C) / 64.0
    for e in range(E_MLP):
        n_e = n_list[e]
        if n_e:
            moe = res2[e]["moe_out"].astype(np.float32)  # [128, 8, ncap]
            out[idx_list[e]] += moe[:, :, :n_e].transpose(
                2, 1, 0).reshape(n_e, C)
    return out.reshape(B, T, C).astype(np.float32)
